# revision 1
# baseline (speedup 1.0000x reference)
"""Trainium2 Bass kernel for a 4-layer GQA transformer LM (nn_CustomLLM_35278861369705).

Sharding: sequence-parallel across 8 cores — 2 batch groups x 4 sequence chunks
of 256 tokens. Activations kept transposed [feature, token] on device.
Per layer: RMSNorm (ones-matmul partition reduction), fused-rope QKV,
group-local AllGather of K/V, masked full-kv attention (uniform SPMD program;
per-core mask data), SwiGLU MLP with PSUM-resident down-proj accumulators.
Final AllGather of hidden states + vocab-sharded tied LM head.
All matmuls run as float32r (full-rate fp32 storage, ~1e-4 rounding).
"""
import numpy as np

import concourse.bass as bass
import concourse.mybir as mybir
import concourse.tile as tile
from concourse import bacc
from concourse.bass_utils import run_bass_kernel_spmd

V, H, NH, KVH, I, L, S, B = 32000, 1024, 16, 4, 4096, 4, 1024, 2
HD = 64
THETA = 10000.0
EPS = 1e-5
T = 256            # tokens per core
NCORE = 8
GROUPS = [[0, 1, 2, 3], [4, 5, 6, 7]]
VSH = V // 4       # vocab shard per core (within its 4-core group)
KT = H // 128      # 8
IT = I // 128      # 32
NVC = 16           # vocab chunks per core
VC = VSH // NVC    # 500

F32 = mybir.dt.float32
F32R = mybir.dt.float32r
AF = mybir.ActivationFunctionType

_CACHE = {}


def build_program(debug_layers=False, single_core=False, repeat=1):
    nc = bacc.Bacc("TRN2", target_bir_lowering=False, debug=False,
                   num_devices=1 if single_core else NCORE)

    # ---------------- I/O ----------------
    x0T = nc.dram_tensor("x0T", [H, T], F32, kind="ExternalInput").ap()
    cos2 = nc.dram_tensor("cos2", [128, T], F32, kind="ExternalInput").ap()
    sin2 = nc.dram_tensor("sin2", [128, T], F32, kind="ExternalInput").ap()
    ropeR = nc.dram_tensor("ropeR", [128, 128], F32, kind="ExternalInput").ap()
    ones_in = nc.dram_tensor("ones_in", [128, 128], F32, kind="ExternalInput").ap()
    mask_in = nc.dram_tensor("mask", [8, 128, T], F32, kind="ExternalInput").ap()
    embT = nc.dram_tensor("embT", [H, VSH], F32, kind="ExternalInput").ap()
    wq_d, wk_d, wv_d, wo_d, wg_d, wu_d, wd_d = [], [], [], [], [], [], []
    for l in range(L):
        wq_d.append(nc.dram_tensor(f"wq{l}", [H, H], F32, kind="ExternalInput").ap())
        wk_d.append(nc.dram_tensor(f"wk{l}", [H, KVH * HD], F32, kind="ExternalInput").ap())
        wv_d.append(nc.dram_tensor(f"wv{l}", [H, KVH * HD], F32, kind="ExternalInput").ap())
        wo_d.append(nc.dram_tensor(f"wo{l}", [H, H], F32, kind="ExternalInput").ap())
        wg_d.append(nc.dram_tensor(f"wg{l}", [H, I], F32, kind="ExternalInput").ap())
        wu_d.append(nc.dram_tensor(f"wu{l}", [H, I], F32, kind="ExternalInput").ap())
        wd_d.append(nc.dram_tensor(f"wd{l}", [I, H], F32, kind="ExternalInput").ap())
    logits = nc.dram_tensor("logits", [S, VSH], F32, kind="ExternalOutput").ap()
    dbg = []
    dbgs = {}
    if debug_layers:
        for l in range(L):
            dbg.append(nc.dram_tensor(f"dbg_x{l}", [H, T], F32, kind="ExternalOutput").ap())
        dbgs["h"] = nc.dram_tensor("dbg_h", [H, T], F32, kind="ExternalOutput").ap()
        dbgs["q"] = nc.dram_tensor("dbg_q", [H, T], F32, kind="ExternalOutput").ap()
        dbgs["k"] = nc.dram_tensor("dbg_k", [256, T], F32, kind="ExternalOutput").ap()
        dbgs["v"] = nc.dram_tensor("dbg_v", [256, T], F32, kind="ExternalOutput").ap()
        dbgs["o"] = nc.dram_tensor("dbg_o", [H, T], F32, kind="ExternalOutput").ap()
        dbgs["xa"] = nc.dram_tensor("dbg_xa", [H, T], F32, kind="ExternalOutput").ap()

    _dma_rr = [0]

    def wdma(dst, srcap):
        eng = (nc.sync, nc.scalar)[_dma_rr[0] % 2]
        _dma_rr[0] += 1
        eng.dma_start(dst, srcap)

    with tile.TileContext(nc) as tc:
        with (
            tc.tile_pool(name="const", bufs=1) as cpool,
            tc.tile_pool(name="xres", bufs=1) as xpool,
            tc.tile_pool(name="hnorm", bufs=2) as hpool,
            tc.tile_pool(name="sqp", bufs=1) as sqpool,
            tc.tile_pool(name="tmps", bufs=3) as tpool,
            tc.tile_pool(name="dram", bufs=2, space="DRAM") as dpool,
        ):
            # ---- persistent constants ----
            cos_sb = cpool.tile([128, T], F32, tag="cos")
            sin_sb = cpool.tile([128, T], F32, tag="sin")
            nc.sync.dma_start(cos_sb[:], cos2[:])
            nc.sync.dma_start(sin_sb[:], sin2[:])
            ropeR_sb = cpool.tile([128, 128], F32R, tag="ropeR")
            nc.sync.dma_start(ropeR_sb[:], ropeR.bitcast(F32R))
            ones_sb = cpool.tile([128, 128], F32R, tag="ones")
            nc.sync.dma_start(ones_sb[:], ones_in.bitcast(F32R))
            mask_sb = cpool.tile([128, 8, T], F32, tag="mask")
            nc.sync.dma_start(mask_sb[:], mask_in.rearrange("j p t -> p j t"))

            # ---- residual stream ----
            xT = xpool.tile([128, KT, T], F32, tag="xT")
            nc.sync.dma_start(xT[:], x0T.rearrange("(kt p) t -> p kt t", p=128))

            def rmsnorm(src):
                """src: [128, KT, T] f32 -> hT [128, KT, T] f32r (no norm weight:
                weights are folded into the following matmul weights on host)."""
                sq = sqpool.tile([128, KT, T], F32R, tag="sq")
                nc.vector.tensor_mul(out=sq[:], in0=src[:], in1=src[:])
                with tc.tile_pool(name="psnorm", bufs=1, space="PSUM") as pp:
                    ps = pp.tile([128, T], F32, tag="ps_norm")
                    for kt in range(KT):
                        nc.tensor.matmul(ps[:], ones_sb[:], sq[:, kt],
                                         start=(kt == 0), stop=(kt == KT - 1))
                    ms = tpool.tile([128, T], F32, tag="ms")
                    nc.scalar.activation(ms[:], ps[:], AF.Copy, bias=EPS, scale=1.0 / H)
                rcp = tpool.tile([128, T], F32, tag="rcp")
                nc.vector.reciprocal(rcp[:], ms[:])
                inv = tpool.tile([128, T], F32, tag="inv")
                nc.scalar.activation(inv[:], rcp[:], AF.Sqrt)
                hT = hpool.tile([128, KT, T], F32R, tag="h")
                nc.vector.tensor_mul(out=hT[:], in0=src[:],
                                     in1=inv[:, None, :].to_broadcast((128, KT, T)))
                return hT

            # =================== layers ===================
            layer_scope = (
                tc.tile_pool(name="acts", bufs=1),
                tc.tile_pool(name="weights", bufs=8),
                tc.tile_pool(name="wop", bufs=2),
                tc.tile_pool(name="wrhs", bufs=2),
            )
            apool, wpool, wopool, wrpool = [p.__enter__() for p in layer_scope]
            for li in range(L * repeat):
                l = li % L
                with nc.named_scope(f"layer{li}_qkv"):
                    hT = rmsnorm(xT)
                    qT = apool.tile([128, KT, T], F32R, tag="qT")
                    kT_loc = apool.tile([128, 2, T], F32, tag="kT_loc")
                    v_loc = apool.tile([128, 2, T], F32, tag="v_loc")

                    with tc.tile_pool(name="psqkv", bufs=2, space="PSUM") as pq:
                        def proj_rope(w_dram, m, out_sl):
                            """project one 128-feature slice and apply rope into out_sl."""
                            wt = wpool.tile([128, KT, 128], F32R, tag="w_h")
                            wsrc = w_dram.rearrange("(kt p) f -> p kt f", p=128)
                            for hh_ in range(2):
                                wdma(wt[:, hh_ * 4:(hh_ + 1) * 4, :],
                                     wsrc[:, hh_ * 4:(hh_ + 1) * 4,
                                          m * 128:(m + 1) * 128].bitcast(F32R))
                            ps = pq.tile([128, T], F32, tag="ps_qkv")
                            for kt in range(KT):
                                nc.tensor.matmul(ps[:], wt[:, kt], hT[:, kt],
                                                 start=(kt == 0), stop=(kt == KT - 1))
                            raw = tpool.tile([128, T], F32R, tag="qraw")
                            nc.vector.tensor_copy(out=raw[:], in_=ps[:])
                            rot = pq.tile([128, T], F32, tag="ps_rot")
                            nc.tensor.matmul(rot[:], ropeR_sb[:], raw[:],
                                             start=True, stop=True)
                            tcs = tpool.tile([128, T], F32, tag="tcos")
                            nc.vector.tensor_mul(out=tcs[:], in0=ps[:], in1=cos_sb[:])
                            tsn = tpool.tile([128, T], F32, tag="tsin")
                            nc.vector.tensor_mul(out=tsn[:], in0=rot[:], in1=sin_sb[:])
                            nc.vector.tensor_add(out=out_sl, in0=tcs[:], in1=tsn[:])

                        for m in range(KT):
                            proj_rope(wq_d[l], m, qT[:, m, :])
                        for m in range(2):
                            proj_rope(wk_d[l], m, kT_loc[:, m, :])
                        # v: natural layout [tok, feat]
                        wvt = wrpool.tile([128, KT, 256], F32R, tag="w_v")
                        for hh_ in range(2):
                            wdma(wvt[:, hh_ * 4:(hh_ + 1) * 4, :],
                                 wv_d[l].rearrange("(kt p) f -> p kt f", p=128)
                                 [:, hh_ * 4:(hh_ + 1) * 4, :].bitcast(F32R))
                        for tt in range(2):
                            psv = pq.tile([128, 256], F32, tag="ps_v")
                            for kt in range(KT):
                                nc.tensor.matmul(psv[:], hT[:, kt, tt * 128:(tt + 1) * 128],
                                                 wvt[:, kt],
                                                 start=(kt == 0), stop=(kt == KT - 1))
                            nc.vector.tensor_copy(out=v_loc[:, tt, :], in_=psv[:])

                    if debug_layers and li == 0:
                        nc.sync.dma_start(
                            dbgs["h"].rearrange("(kt p) t -> p kt t", p=128),
                            hT.bitcast(F32))
                        nc.sync.dma_start(
                            dbgs["q"].rearrange("(kt p) t -> p kt t", p=128),
                            qT.bitcast(F32))
                        nc.sync.dma_start(
                            dbgs["k"].rearrange("(s p) t -> p s t", p=128), kT_loc[:])
                        nc.sync.dma_start(
                            dbgs["v"].rearrange("(s p) t -> p s t", p=128), v_loc[:])

                    # ---- AllGather K/V within group ----
                    cc_in = dpool.tile([4 * 128, T], F32, tag="cc_in")
                    cc_in_r = cc_in.rearrange("(s p) t -> p s t", p=128)
                    nc.sync.dma_start(cc_in_r[:, 0:2, :], kT_loc[:])
                    nc.sync.dma_start(cc_in_r[:, 2:4, :], v_loc[:])
                    cc_out = dpool.tile([4 * 4 * 128, T], F32, tag="cc_out")
                    if single_core:
                        for rr in range(4):
                            nc.sync.dma_start(cc_out[rr * 512:(rr + 1) * 512], cc_in[:])
                    else:
                        nc.gpsimd.collective_compute(
                            "AllGather", mybir.AluOpType.bypass,
                            ins=[cc_in.opt()], outs=[cc_out.opt()],
                            replica_groups=GROUPS)
                    cc_r = cc_out.rearrange("(c s p) t -> p c s t", c=4, s=4)

                with nc.named_scope(f"layer{li}_attn"):
                    kg = []
                    vg = []
                    for g in range(KVH):
                        # duplicate k rows into both partition halves so matmuls
                        # with q heads at base 0 or 64 both have matching bases
                        kgt = apool.tile([128, 4, T], F32R, tag=f"kg{g}")
                        src = cc_r[64 * (g % 2):64 * (g % 2) + 64, :, g // 2, :].bitcast(F32R)
                        nc.sync.dma_start(kgt[0:64], src)
                        nc.sync.dma_start(kgt[64:128], src)
                        kg.append(kgt)
                        vgt = apool.tile([128, 4, 2, HD], F32R, tag=f"vg{g}")
                        for tt in range(2):
                            nc.sync.dma_start(
                                vgt[:, :, tt, :],
                                cc_r[:, :, 2 + tt, g * HD:(g + 1) * HD].bitcast(F32R))
                        vg.append(vgt)

                    oT = apool.tile([64, NH, T], F32R, tag="oT")
                    with (
                        tc.tile_pool(name="psatt", bufs=2, space="PSUM") as pa,
                        tc.tile_pool(name="pexp", bufs=4) as epool,
                    ):
                        for h in range(NH):
                            g = h // 4
                            q_sl = qT[64 * (h % 2):64 * (h % 2) + 64, h // 2, :]
                            base = 64 * (h % 2)
                            pjs = []
                            for c in range(4):
                                ps_s = pa.tile([128, 2, T], F32, tag="ps_s")
                                for mt in range(2):
                                    nc.tensor.matmul(
                                        ps_s[:, mt, :],
                                        kg[g][base:base + 64, c, mt * 128:(mt + 1) * 128],
                                        q_sl, start=True, stop=True)
                                e1 = epool.tile([128, 2, T], F32, tag="e1")
                                nc.scalar.activation(e1[:], ps_s[:], AF.Exp, scale=0.125)
                                pj = epool.tile([128, 2, T], F32R, tag="pj")
                                nc.vector.tensor_mul(out=pj[:], in0=e1[:],
                                                     in1=mask_sb[:, 2 * c:2 * c + 2, :])
                                pjs.append(pj)
                            ps_sum = pa.tile([128, T], F32, tag="ps_sum")
                            ps_o = pa.tile([64, T], F32, tag="ps_o")
                            for c in range(4):
                                for tt in range(2):
                                    j = 2 * c + tt
                                    nc.tensor.matmul(ps_sum[:], ones_sb[:],
                                                     pjs[c][:, tt, :],
                                                     start=(j == 0), stop=(j == 7))
                                    nc.tensor.matmul(ps_o[:], vg[g][:, c, tt, :],
                                                     pjs[c][:, tt, :],
                                                     start=(j == 0), stop=(j == 7))
                            invb = epool.tile([128, T], F32, tag="invb")
                            nc.vector.reciprocal(invb[:], ps_sum[:])
                            nc.vector.tensor_mul(out=oT[:, h, :], in0=ps_o[:],
                                                 in1=invb[0:64, :])

                    # ---- o-projection + residual ----
                    wo_r = wo_d[l].rearrange("(hh p) f -> p hh f", p=64)
                    with tc.tile_pool(name="psoproj", bufs=2, space="PSUM") as po:
                        for m in range(KT):
                            wot = wopool.tile([64, NH, 128], F32R, tag="w_o")
                            for hh_ in range(2):
                                wdma(wot[:, hh_ * 8:(hh_ + 1) * 8, :],
                                     wo_r[:, hh_ * 8:(hh_ + 1) * 8,
                                          m * 128:(m + 1) * 128].bitcast(F32R))
                            ps = po.tile([128, T], F32, tag="ps_op")
                            for hh in range(NH):
                                nc.tensor.matmul(ps[:], wot[:, hh], oT[:, hh, :],
                                                 start=(hh == 0), stop=(hh == NH - 1))
                            nc.vector.tensor_add(out=xT[:, m, :], in0=xT[:, m, :], in1=ps[:])

                    if debug_layers and li == 0:
                        nc.sync.dma_start(
                            dbgs["o"].rearrange("(hh p) t -> p hh t", p=64),
                            oT.bitcast(F32))
                        nc.sync.dma_start(
                            dbgs["xa"].rearrange("(kt p) t -> p kt t", p=128), xT[:])

                with nc.named_scope(f"layer{li}_mlp"):
                    h2T = rmsnorm(xT)
                    with (
                        tc.tile_pool(name="psmlpd", bufs=1, space="PSUM") as pmd,
                        tc.tile_pool(name="psmlp", bufs=2, space="PSUM") as pm,
                    ):
                        ps_d = [pmd.tile([128, 2, T], F32, tag=f"ps_d{i}", name=f"ps_d{i}")
                                for i in range(4)]
                        for f in range(IT):
                            wgt = wpool.tile([128, KT, 128], F32R, tag="w_h")
                            for hh_ in range(2):
                                wdma(wgt[:, hh_ * 4:(hh_ + 1) * 4, :],
                                     wg_d[l].rearrange("(kt p) f -> p kt f", p=128)
                                     [:, hh_ * 4:(hh_ + 1) * 4,
                                      f * 128:(f + 1) * 128].bitcast(F32R))
                            ps_g = pm.tile([128, T], F32, tag="ps_g")
                            for kt in range(KT):
                                nc.tensor.matmul(ps_g[:], wgt[:, kt], h2T[:, kt],
                                                 start=(kt == 0), stop=(kt == KT - 1))
                            wut = wpool.tile([128, KT, 128], F32R, tag="w_h")
                            for hh_ in range(2):
                                wdma(wut[:, hh_ * 4:(hh_ + 1) * 4, :],
                                     wu_d[l].rearrange("(kt p) f -> p kt f", p=128)
                                     [:, hh_ * 4:(hh_ + 1) * 4,
                                      f * 128:(f + 1) * 128].bitcast(F32R))
                            ps_u = pm.tile([128, T], F32, tag="ps_u")
                            for kt in range(KT):
                                nc.tensor.matmul(ps_u[:], wut[:, kt], h2T[:, kt],
                                                 start=(kt == 0), stop=(kt == KT - 1))
                            silu = tpool.tile([128, T], F32, tag="silu")
                            nc.scalar.activation(silu[:], ps_g[:], AF.Silu)
                            gu = tpool.tile([128, T], F32R, tag="gu")
                            nc.vector.tensor_mul(out=gu[:], in0=silu[:], in1=ps_u[:])
                            wdt = wpool.tile([128, KT, 128], F32R, tag="w_h")
                            wdsrc = wd_d[l].rearrange("(ft p) f -> p ft f", p=128)[:, f, :]
                            for hh_ in range(2):
                                wdma(wdt[:, hh_ * 4:(hh_ + 1) * 4, :],
                                     wdsrc[:, hh_ * 512:(hh_ + 1) * 512]
                                     .rearrange("p (a b) -> p a b", a=4).bitcast(F32R))
                            for m in range(KT):
                                # start=True clears the WHOLE bank's has_written,
                                # so only the first matmul touching each bank may
                                # set it; the odd slice's first write then stores
                                # (has_written=0) and later writes accumulate.
                                nc.tensor.matmul(ps_d[m // 2][:, m % 2, :],
                                                 wdt[:, m], gu[:],
                                                 start=(f == 0 and m % 2 == 0),
                                                 stop=(f == IT - 1),
                                                 skip_group_check=True)
                        for m in range(KT):
                            nc.vector.tensor_add(out=xT[:, m, :], in0=xT[:, m, :],
                                                 in1=ps_d[m // 2][:, m % 2, :])
                if debug_layers and repeat == 1:
                    nc.sync.dma_start(
                        dbg[l].rearrange("(kt p) t -> p kt t", p=128), xT[:])

            for p in reversed(layer_scope):
                p.__exit__(None, None, None)

            # =================== LM head ===================
            with nc.named_scope("lm_head"):
                hfT = rmsnorm(xT)
                cc2_in = dpool.tile([H, T], F32, tag="cc2_in")
                nc.sync.dma_start(cc2_in.rearrange("(kt p) t -> p kt t", p=128),
                                  hfT.bitcast(F32))
                cc2_out = dpool.tile([4 * H, T], F32, tag="cc2_out")
                if single_core:
                    for rr in range(4):
                        nc.sync.dma_start(cc2_out[rr * H:(rr + 1) * H], cc2_in[:])
                else:
                    nc.gpsimd.collective_compute(
                        "AllGather", mybir.AluOpType.bypass,
                        ins=[cc2_in.opt()], outs=[cc2_out.opt()],
                        replica_groups=GROUPS)
                cc2_r = cc2_out.rearrange("(c kt p) t -> p c kt t", c=4, kt=KT)

                with (
                    tc.tile_pool(name="hall", bufs=1) as hallp,
                    tc.tile_pool(name="embp", bufs=2) as embp,
                    tc.tile_pool(name="lsbp", bufs=4) as lsbp,
                    tc.tile_pool(name="pslm", bufs=4, space="PSUM") as plm,
                ):
                    ha = []
                    for m8 in range(8):
                        hat = hallp.tile([128, KT, 128], F32R, tag=f"ha{m8}")
                        nc.sync.dma_start(
                            hat[:],
                            cc2_r[:, m8 // 2, :, 128 * (m8 % 2):128 * (m8 % 2) + 128]
                            .bitcast(F32R))
                        ha.append(hat)
                    embT_r = embT.rearrange("(kt p) v -> p kt v", p=128)
                    for vc in range(NVC):
                        et = embp.tile([128, KT, VC], F32R, tag="emb")
                        for kt_ in range(KT):
                            wdma(et[:, kt_, :],
                                 embT_r[:, kt_, vc * VC:(vc + 1) * VC].bitcast(F32R))
                        for m8 in range(8):
                            ps = plm.tile([128, VC], F32, tag="ps_lm")
                            for kt in range(KT):
                                nc.tensor.matmul(ps[:], ha[m8][:, kt], et[:, kt],
                                                 start=(kt == 0), stop=(kt == KT - 1))
                            lsb = lsbp.tile([128, VC], F32, tag="lsb")
                            nc.any.tensor_copy(out=lsb[:], in_=ps[:])
                            nc.sync.dma_start(
                                logits[m8 * 128:(m8 + 1) * 128, vc * VC:(vc + 1) * VC],
                                lsb[:])

    nc.finalize()
    return nc


# ---------------- host side ----------------

def _host_prep(inputs):
    """Build per-core input maps from full inputs."""
    ids = np.asarray(inputs["input_ids"])
    embed = np.asarray(inputs["embed"], dtype=np.float32)
    n1 = np.asarray(inputs["norm1_w"], dtype=np.float32)
    n2 = np.asarray(inputs["norm2_w"], dtype=np.float32)
    nf = np.asarray(inputs["final_norm_w"], dtype=np.float32)

    inv_freq = 1.0 / (THETA ** (np.arange(0, HD, 2, dtype=np.float64) / HD))
    R64 = np.zeros((HD, HD), np.float32)
    for i in range(32):
        R64[i, i + 32] = -1.0
        R64[i + 32, i] = 1.0
    Rblk = np.zeros((128, 128), np.float32)
    Rblk[:64, :64] = R64
    Rblk[64:, 64:] = R64
    ropeR = np.ascontiguousarray(Rblk.T)
    ones128 = np.ones((128, 128), np.float32)

    # fold norm weights into following matmul weights (they are ones in practice,
    # but fold anyway for generality)
    common = {"ropeR": ropeR, "ones_in": ones128}
    for l in range(L):
        common[f"wq{l}"] = np.ascontiguousarray(n1[l][:, None] * np.asarray(inputs["wq"][l], np.float32))
        common[f"wk{l}"] = np.ascontiguousarray(n1[l][:, None] * np.asarray(inputs["wk"][l], np.float32))
        common[f"wv{l}"] = np.ascontiguousarray(n1[l][:, None] * np.asarray(inputs["wv"][l], np.float32))
        common[f"wo{l}"] = np.ascontiguousarray(np.asarray(inputs["wo"][l], np.float32))
        common[f"wg{l}"] = np.ascontiguousarray(n2[l][:, None] * np.asarray(inputs["w_gate"][l], np.float32))
        common[f"wu{l}"] = np.ascontiguousarray(n2[l][:, None] * np.asarray(inputs["w_up"][l], np.float32))
        common[f"wd{l}"] = np.ascontiguousarray(np.asarray(inputs["w_down"][l], np.float32))

    in_maps = []
    for core in range(NCORE):
        b, qc = core // 4, core % 4
        pos = np.arange(T, dtype=np.float64) + qc * T
        freqs = np.outer(pos, inv_freq)
        emb = np.concatenate([freqs, freqs], axis=-1)
        cosT = np.cos(emb).T.astype(np.float32)
        sinT = np.sin(emb).T.astype(np.float32)
        mask = np.zeros((8, 128, T), np.float32)
        kvpos = np.arange(1024).reshape(8, 128)
        qpos = (np.arange(T) + qc * T)
        for j in range(8):
            mask[j] = (kvpos[j][:, None] <= qpos[None, :]).astype(np.float32)
        x0T = np.ascontiguousarray(embed[ids[b, qc * T:(qc + 1) * T]].T)
        vbase = (core % 4) * VSH
        embT_shard = np.ascontiguousarray((nf[:, None] * embed[vbase:vbase + VSH].T))
        m = dict(common)
        m.update({
            "x0T": x0T.astype(np.float32),
            "cos2": np.ascontiguousarray(np.tile(cosT, (2, 1))),
            "sin2": np.ascontiguousarray(np.tile(sinT, (2, 1))),
            "mask": mask,
            "embT": embT_shard.astype(np.float32),
        })
        in_maps.append(m)
    return in_maps


def _get_program(debug_layers=False):
    key = ("prog", debug_layers)
    if key not in _CACHE:
        _CACHE[key] = build_program(debug_layers)
    return _CACHE[key]


def run(inputs, debug_layers=False, trace=False):
    nc = _get_program(debug_layers)
    in_maps = _host_prep(inputs)
    res = run_bass_kernel_spmd(nc, in_maps, core_ids=list(range(NCORE)), trace=trace)
    out = np.zeros((B, S, V), np.float32)
    for b in range(B):
        out[b] = np.concatenate(
            [res.results[4 * b + i]["logits"] for i in range(4)], axis=1)
    return out, res


def kernel(**inputs) -> np.ndarray:
    out, _ = run(inputs)
    return out



# revision 9
# speedup vs baseline: 1.4127x; 1.4127x over previous
"""Trainium2 Bass kernel for a 4-layer GQA transformer LM (nn_CustomLLM_35278861369705).

Sharding: sequence-parallel across 8 cores - 2 batch groups x 4 sequence chunks
of 256 tokens. Activations kept transposed [feature, token] on device.
All matmul operands are bf16 (fp32 PSUM accumulation, fp32 residual stream).
Weights are pre-laid out on the host in tile-ready form so each weight tensor
loads with a single large fully-contiguous DMA; MLP weights stream in 16
chunks of 1.5MB (gate+up+down packed) with triple buffering.
Per layer: RMSNorm (ones-matmul partition reduction), fused-rope QKV,
split K / V group AllGathers (K gathered while Q/V project), masked full-kv
attention, SwiGLU MLP with PSUM-resident down-proj accumulators.
Final AllGather of hidden states + vocab-sharded tied LM head with SBUF-staged
logits written back in 2MB tiles.
"""
import numpy as np
import ml_dtypes

import concourse.bass as bass
import concourse.mybir as mybir
import concourse.tile as tile
from concourse import bacc
from concourse.bass_utils import run_bass_kernel_spmd

V, H, NH, KVH, I, L, S, B = 32000, 1024, 16, 4, 4096, 4, 1024, 2
HD = 64
THETA = 10000.0
EPS = 1e-5
T = 256            # tokens per core
NCORE = 8
GROUPS = [[0, 1, 2, 3], [4, 5, 6, 7]]
VSH = V // 4       # vocab shard per core (within its 4-core group)
KT = H // 128      # 8
NC2 = 16           # MLP I-chunks per layer (256 feats each)
NVC = 16           # vocab chunks per core
VC = VSH // NVC    # 500

F32 = mybir.dt.float32
BF16 = mybir.dt.bfloat16
AF = mybir.ActivationFunctionType
BF16NP = ml_dtypes.bfloat16

# MLP chunk free-dim offsets (elements within the flat [128, 6144] tile)
GOFF = 0           # gate region:  (kt, fi) -> (kt*2+fi)*128
UOFF = 2048        # up region:    (kt, fi) -> UOFF + (kt*2+fi)*128
DOFF = 4096        # down region:  (ci, m)  -> DOFF + (ci*8+m)*128

_CACHE = {}


def build_program():
    nc = bacc.Bacc("TRN2", target_bir_lowering=False, debug=False,
                   num_devices=NCORE)

    # ---------------- I/O ----------------
    x0T = nc.dram_tensor("x0T", [128, KT, T], F32, kind="ExternalInput").ap()
    cos2 = nc.dram_tensor("cos2", [128, T], F32, kind="ExternalInput").ap()
    sin2 = nc.dram_tensor("sin2", [128, T], F32, kind="ExternalInput").ap()
    ropeR = nc.dram_tensor("ropeR", [128, 128], BF16, kind="ExternalInput").ap()
    ones_in = nc.dram_tensor("ones_in", [128, 128], BF16, kind="ExternalInput").ap()
    mask_in = nc.dram_tensor("mask", [128, 8, T], BF16, kind="ExternalInput").ap()
    embT = nc.dram_tensor("embT", [NVC, 128, KT, VC], BF16, kind="ExternalInput").ap()
    wq_d, wk_d, wv_d, wo_d, wm_d = [], [], [], [], []
    for l in range(L):
        wq_d.append(nc.dram_tensor(f"wq{l}", [128, KT, KT, 128], BF16,
                                   kind="ExternalInput").ap())   # [p, m, kt, of]
        wk_d.append(nc.dram_tensor(f"wk{l}", [128, 2, KT, 128], BF16,
                                   kind="ExternalInput").ap())   # [p, m, kt, of]
        wv_d.append(nc.dram_tensor(f"wv{l}", [128, KT, 256], BF16,
                                   kind="ExternalInput").ap())   # [p, kt, of]
        wo_d.append(nc.dram_tensor(f"wo{l}", [128, KT, KT, 128], BF16,
                                   kind="ExternalInput").ap())   # [p, j, m, of]
        wm_d.append(nc.dram_tensor(f"wm{l}", [NC2, 128, 6144], BF16,
                                   kind="ExternalInput").ap())   # packed g/u/d
    logits_t = nc.dram_tensor("logits_t", [NVC, 128, 8, VC], F32,
                              kind="ExternalOutput").ap()

    with tile.TileContext(nc) as tc:
        with (
            tc.tile_pool(name="const", bufs=1) as cpool,
            tc.tile_pool(name="xres", bufs=1) as xpool,
            tc.tile_pool(name="hnorm", bufs=2) as hpool,
            tc.tile_pool(name="sqp", bufs=2) as sqpool,
            tc.tile_pool(name="tmps", bufs=3) as tpool,
            tc.tile_pool(name="dram", bufs=2, space="DRAM") as dpool,
        ):
            # ---- persistent constants ----
            cos_sb = cpool.tile([128, T], F32, tag="cos")
            sin_sb = cpool.tile([128, T], F32, tag="sin")
            nc.sync.dma_start(cos_sb[:], cos2[:])
            nc.sync.dma_start(sin_sb[:], sin2[:])
            ropeR_sb = cpool.tile([128, 128], BF16, tag="ropeR")
            nc.sync.dma_start(ropeR_sb[:], ropeR[:])
            ones_sb = cpool.tile([128, 128], BF16, tag="ones")
            nc.sync.dma_start(ones_sb[:], ones_in[:])
            mask_sb = cpool.tile([128, 8, T], BF16, tag="mask")
            nc.sync.dma_start(mask_sb[:], mask_in[:])

            # ---- residual stream ----
            xT = xpool.tile([128, KT, T], F32, tag="xT")
            nc.sync.dma_start(xT[:], x0T[:])

            def rmsnorm(src):
                """src: [128, KT, T] f32 -> hT [128, KT, T] bf16 (norm weights
                are folded into the following matmul weights on host)."""
                sq = sqpool.tile([128, KT, T], BF16, tag="sq")
                nc.vector.tensor_mul(out=sq[:], in0=src[:], in1=src[:])
                with tc.tile_pool(name="psnorm", bufs=1, space="PSUM") as pp:
                    ps = pp.tile([128, T], F32, tag="ps_norm")
                    for kt in range(KT):
                        nc.tensor.matmul(ps[:], ones_sb[:], sq[:, kt],
                                         start=(kt == 0), stop=(kt == KT - 1))
                    ms = tpool.tile([128, T], F32, tag="ms")
                    nc.scalar.activation(ms[:], ps[:], AF.Copy, bias=EPS, scale=1.0 / H)
                rcp = tpool.tile([128, T], F32, tag="rcp")
                nc.vector.reciprocal(rcp[:], ms[:])
                inv = tpool.tile([128, T], F32, tag="inv")
                nc.scalar.activation(inv[:], rcp[:], AF.Sqrt)
                hT = hpool.tile([128, KT, T], BF16, tag="h")
                nc.vector.tensor_mul(out=hT[:], in0=src[:],
                                     in1=inv[:, None, :].to_broadcast((128, KT, T)))
                return hT

            # =================== layers ===================
            layer_scope = (
                tc.tile_pool(name="wqp", bufs=1),
                tc.tile_pool(name="wkp", bufs=2),
                tc.tile_pool(name="wvp", bufs=2),
                tc.tile_pool(name="wop", bufs=1),
                tc.tile_pool(name="wmp", bufs=3),
                tc.tile_pool(name="acts", bufs=1),
            )
            wqpool, wkpool, wvpool, wopool, wmpool, apool = \
                [p.__enter__() for p in layer_scope]
            for l in range(L):
                with nc.named_scope(f"layer{l}_qkv"):
                    hT = rmsnorm(xT)
                    # single big weight DMAs (fully contiguous layouts)
                    wqt = wqpool.tile([128, KT, KT, 128], BF16, tag="w_q")
                    nc.sync.dma_start(wqt[:], wq_d[l][:])
                    wkt = wkpool.tile([128, 2, KT, 128], BF16, tag="w_k")
                    nc.sync.dma_start(wkt[:], wk_d[l][:])
                    wvt = wvpool.tile([128, KT, 256], BF16, tag="w_v")
                    nc.sync.dma_start(wvt[:], wv_d[l][:])

                    qT = apool.tile([128, KT, T], BF16, tag="qT")
                    kT_loc = apool.tile([128, 2, T], BF16, tag="kT_loc")
                    v_loc = apool.tile([128, 2, T], BF16, tag="v_loc")

                    with tc.tile_pool(name="psqkv", bufs=2, space="PSUM") as pq:
                        def proj_rope(wt_sl, out_sl):
                            """one 128-feature projection slice + rope -> out_sl."""
                            ps = pq.tile([128, T], F32, tag="ps_qkv")
                            for kt in range(KT):
                                nc.tensor.matmul(ps[:], wt_sl[:, kt], hT[:, kt],
                                                 start=(kt == 0), stop=(kt == KT - 1))
                            raw = tpool.tile([128, T], BF16, tag="qraw")
                            nc.vector.tensor_copy(out=raw[:], in_=ps[:])
                            rot = pq.tile([128, T], F32, tag="ps_rot")
                            nc.tensor.matmul(rot[:], ropeR_sb[:], raw[:],
                                             start=True, stop=True)
                            tcs = tpool.tile([128, T], F32, tag="tcos")
                            nc.vector.tensor_mul(out=tcs[:], in0=ps[:], in1=cos_sb[:])
                            tsn = tpool.tile([128, T], F32, tag="tsin")
                            nc.vector.tensor_mul(out=tsn[:], in0=rot[:], in1=sin_sb[:])
                            nc.vector.tensor_add(out=out_sl, in0=tcs[:], in1=tsn[:])

                        # K first so its AllGather overlaps Q/V compute
                        for m in range(2):
                            proj_rope(wkt[:, m], kT_loc[:, m, :])
                        cck_in = dpool.tile([2 * 128, T], BF16, tag="cck_in")
                        nc.scalar.dma_start(
                            cck_in.rearrange("(s p) t -> p s t", p=128), kT_loc[:])
                        cck_out = dpool.tile([4 * 2 * 128, T], BF16, tag="cck_out")
                        nc.gpsimd.collective_compute(
                            "AllGather", mybir.AluOpType.bypass,
                            ins=[cck_in.opt()], outs=[cck_out.opt()],
                            replica_groups=GROUPS)
                        cck_r = cck_out.rearrange("(c s p) t -> p c s t", c=4, s=2)

                        for m in range(KT):
                            proj_rope(wqt[:, m], qT[:, m, :])

                        # V in natural [token, feature] layout
                        for tt in range(2):
                            psv = pq.tile([128, T], F32, tag="ps_qkv")
                            for kt in range(KT):
                                nc.tensor.matmul(psv[:], hT[:, kt, tt * 128:(tt + 1) * 128],
                                                 wvt[:, kt],
                                                 start=(kt == 0), stop=(kt == KT - 1))
                            nc.vector.tensor_copy(out=v_loc[:, tt, :], in_=psv[:])
                        ccv_in = dpool.tile([2 * 128, T], BF16, tag="ccv_in")
                        nc.scalar.dma_start(
                            ccv_in.rearrange("(s p) t -> p s t", p=128), v_loc[:])
                        ccv_out = dpool.tile([4 * 2 * 128, T], BF16, tag="ccv_out")
                        nc.gpsimd.collective_compute(
                            "AllGather", mybir.AluOpType.bypass,
                            ins=[ccv_in.opt()], outs=[ccv_out.opt()],
                            replica_groups=GROUPS)
                        ccv_r = ccv_out.rearrange("(c s p) t -> p c s t", c=4, s=2)

                with nc.named_scope(f"layer{l}_attn"):
                    # o-proj weights: issue DMA early so it overlaps attention
                    wot = wopool.tile([128, KT, KT, 128], BF16, tag="w_o")
                    nc.sync.dma_start(wot[:], wo_d[l][:])

                    kg = []
                    for g in range(KVH):
                        # duplicate k rows into both partition halves so matmuls
                        # with q heads at base 0 or 64 both have matching bases
                        kgt = apool.tile([128, 4, T], BF16, tag=f"kg{g}")
                        src = cck_r[64 * (g % 2):64 * (g % 2) + 64, :, g // 2, :]
                        nc.scalar.dma_start(kgt[0:64], src)
                        nc.scalar.dma_start(kgt[64:128], src)
                        kg.append(kgt)
                    # all gathered V in one contiguous SBUF tile; attention
                    # slices it directly as the stationary operand
                    vall = apool.tile([128, 4, 2, T], BF16, tag="vall")
                    nc.scalar.dma_start(vall[:], ccv_r[:])

                    oT = apool.tile([128, KT, T], BF16, tag="oT")
                    with (
                        tc.tile_pool(name="psatt", bufs=3, space="PSUM") as pa,
                        tc.tile_pool(name="pssum", bufs=2, space="PSUM") as pas,
                        tc.tile_pool(name="pexp", bufs=6) as epool,
                    ):
                        for h in range(NH):
                            g = h // 4
                            base = 64 * (h % 2)
                            q_sl = qT[base:base + 64, h // 2, :]
                            pjs = []
                            for c in range(4):
                                ps_s = pa.tile([128, 2, T], F32, tag="ps_s")
                                for mt in range(2):
                                    nc.tensor.matmul(
                                        ps_s[:, mt, :],
                                        kg[g][base:base + 64, c, mt * 128:(mt + 1) * 128],
                                        q_sl, start=True, stop=True)
                                e1 = epool.tile([128, 2, T], BF16, tag="e1")
                                nc.scalar.activation(e1[:], ps_s[:], AF.Exp, scale=0.125)
                                pj = epool.tile([128, 2, T], BF16, tag="pj")
                                nc.vector.tensor_mul(out=pj[:], in0=e1[:],
                                                     in1=mask_sb[:, 2 * c:2 * c + 2, :])
                                pjs.append(pj)
                            ps_sum = pas.tile([128, T], F32, tag="ps_sum")
                            ps_o = pas.tile([64, T], F32, tag="ps_o")
                            for c in range(4):
                                for tt in range(2):
                                    j = 2 * c + tt
                                    nc.tensor.matmul(ps_sum[:], ones_sb[:],
                                                     pjs[c][:, tt, :],
                                                     start=(j == 0), stop=(j == 7))
                                    nc.tensor.matmul(
                                        ps_o[:],
                                        vall[:, c, tt, g * HD:(g + 1) * HD],
                                        pjs[c][:, tt, :],
                                        start=(j == 0), stop=(j == 7))
                            invb = epool.tile([128, T], F32, tag="invb")
                            nc.vector.reciprocal(invb[:], ps_sum[:])
                            nc.vector.tensor_mul(out=oT[base:base + 64, h // 2, :],
                                                 in0=ps_o[:], in1=invb[0:64, :])

                    # ---- o-projection + residual ----
                    with tc.tile_pool(name="psoproj", bufs=2, space="PSUM") as po:
                        for m in range(KT):
                            ps = po.tile([128, T], F32, tag="ps_op")
                            for j in range(KT):
                                nc.tensor.matmul(ps[:], wot[:, j, m], oT[:, j, :],
                                                 start=(j == 0), stop=(j == KT - 1))
                            nc.vector.tensor_add(out=xT[:, m, :], in0=xT[:, m, :],
                                                 in1=ps[:])

                with nc.named_scope(f"layer{l}_mlp"):
                    h2T = rmsnorm(xT)
                    with (
                        tc.tile_pool(name="psmlpd", bufs=1, space="PSUM") as pmd,
                        tc.tile_pool(name="psmlp", bufs=2, space="PSUM") as pm,
                    ):
                        ps_d = [pmd.tile([128, 2, T], F32, tag=f"ps_d{i}", name=f"ps_d{i}")
                                for i in range(4)]
                        for c in range(NC2):
                            wmt = wmpool.tile([128, 6144], BF16, tag="wmlp")
                            nc.sync.dma_start(wmt[:], wm_d[l][c])
                            for fi in range(2):
                                ps_g = pm.tile([128, T], F32, tag="ps_g")
                                for kt in range(KT):
                                    off = GOFF + (kt * 2 + fi) * 128
                                    nc.tensor.matmul(ps_g[:], wmt[:, off:off + 128],
                                                     h2T[:, kt],
                                                     start=(kt == 0), stop=(kt == KT - 1))
                                ps_u = pm.tile([128, T], F32, tag="ps_u")
                                for kt in range(KT):
                                    off = UOFF + (kt * 2 + fi) * 128
                                    nc.tensor.matmul(ps_u[:], wmt[:, off:off + 128],
                                                     h2T[:, kt],
                                                     start=(kt == 0), stop=(kt == KT - 1))
                                silu = tpool.tile([128, T], F32, tag="silu")
                                nc.scalar.activation(silu[:], ps_g[:], AF.Silu)
                                gu = tpool.tile([128, T], BF16, tag="gu")
                                nc.vector.tensor_mul(out=gu[:], in0=silu[:], in1=ps_u[:])
                                first = (c == 0 and fi == 0)
                                last = (c == NC2 - 1 and fi == 1)
                                for m in range(KT):
                                    off = DOFF + (fi * 8 + m) * 128
                                    # start=True clears the WHOLE bank's has_written,
                                    # so only the first matmul touching each bank may
                                    # set it; the odd slice's first write then stores
                                    # (has_written=0) and later writes accumulate.
                                    nc.tensor.matmul(ps_d[m // 2][:, m % 2, :],
                                                     wmt[:, off:off + 128], gu[:],
                                                     start=(first and m % 2 == 0),
                                                     stop=last,
                                                     skip_group_check=True)
                        for m in range(KT):
                            nc.vector.tensor_add(out=xT[:, m, :], in0=xT[:, m, :],
                                                 in1=ps_d[m // 2][:, m % 2, :])

            for p in reversed(layer_scope):
                p.__exit__(None, None, None)

            # =================== LM head ===================
            with nc.named_scope("lm_head"):
                hfT = rmsnorm(xT)
                cc2_in = dpool.tile([H, T], BF16, tag="cc2_in")
                nc.scalar.dma_start(cc2_in.rearrange("(kt p) t -> p kt t", p=128),
                                    hfT[:])
                cc2_out = dpool.tile([4 * H, T], BF16, tag="cc2_out")
                nc.gpsimd.collective_compute(
                    "AllGather", mybir.AluOpType.bypass,
                    ins=[cc2_in.opt()], outs=[cc2_out.opt()],
                    replica_groups=GROUPS)
                cc2_r = cc2_out.rearrange("(c kt p) t -> p c kt t", c=4, kt=KT)

                with (
                    tc.tile_pool(name="hall", bufs=1) as hallp,
                    tc.tile_pool(name="embp", bufs=3) as embp,
                    tc.tile_pool(name="lsbp", bufs=2) as lsbp,
                    tc.tile_pool(name="pslm", bufs=4, space="PSUM") as plm,
                ):
                    ha = []
                    for m8 in range(8):
                        hat = hallp.tile([128, KT, 128], BF16, tag=f"ha{m8}")
                        nc.scalar.dma_start(
                            hat[:],
                            cc2_r[:, m8 // 2, :, 128 * (m8 % 2):128 * (m8 % 2) + 128])
                        ha.append(hat)
                    for vc in range(NVC):
                        et = embp.tile([128, KT, VC], BF16, tag="emb")
                        nc.sync.dma_start(et[:], embT[vc])
                        lsb = lsbp.tile([128, 8, VC], F32, tag="lsb")
                        for m8 in range(8):
                            ps = plm.tile([128, VC], F32, tag="ps_lm")
                            for kt in range(KT):
                                nc.tensor.matmul(ps[:], ha[m8][:, kt], et[:, kt],
                                                 start=(kt == 0), stop=(kt == KT - 1))
                            nc.vector.tensor_copy(out=lsb[:, m8, :], in_=ps[:])
                        nc.sync.dma_start(logits_t[vc], lsb[:])

    nc.finalize()
    return nc


# ---------------- host side ----------------

def _host_prep(inputs):
    """Build per-core input maps from full inputs (tile-ready bf16 layouts)."""
    ids = np.asarray(inputs["input_ids"])
    embed = np.asarray(inputs["embed"], dtype=np.float32)
    n1 = np.asarray(inputs["norm1_w"], dtype=np.float32)
    n2 = np.asarray(inputs["norm2_w"], dtype=np.float32)
    nf = np.asarray(inputs["final_norm_w"], dtype=np.float32)

    inv_freq = 1.0 / (THETA ** (np.arange(0, HD, 2, dtype=np.float64) / HD))
    R64 = np.zeros((HD, HD), np.float32)
    for i in range(32):
        R64[i, i + 32] = -1.0
        R64[i + 32, i] = 1.0
    Rblk = np.zeros((128, 128), np.float32)
    Rblk[:64, :64] = R64
    Rblk[64:, 64:] = R64
    ropeR = np.ascontiguousarray(Rblk.T).astype(BF16NP)
    ones128 = np.ones((128, 128), BF16NP)

    def bf(x):
        return np.ascontiguousarray(x).astype(BF16NP)

    common = {"ropeR": ropeR, "ones_in": ones128}
    for l in range(L):
        wq = n1[l][:, None] * np.asarray(inputs["wq"][l], np.float32)    # [H, H]
        common[f"wq{l}"] = bf(wq.reshape(KT, 128, KT, 128).transpose(1, 2, 0, 3))
        wk = n1[l][:, None] * np.asarray(inputs["wk"][l], np.float32)    # [H, 256]
        common[f"wk{l}"] = bf(wk.reshape(KT, 128, 2, 128).transpose(1, 2, 0, 3))
        wv = n1[l][:, None] * np.asarray(inputs["wv"][l], np.float32)    # [H, 256]
        common[f"wv{l}"] = bf(wv.reshape(KT, 128, 256).transpose(1, 0, 2))
        wo = np.asarray(inputs["wo"][l], np.float32)                     # [H, H]
        # in-feature index = 64*(2j+s)+d -> partition p = 64*s+d, free j
        common[f"wo{l}"] = bf(wo.reshape(KT, 2, 64, KT, 128).transpose(1, 2, 0, 3, 4)
                              .reshape(128, KT, KT, 128))
        wg = n2[l][:, None] * np.asarray(inputs["w_gate"][l], np.float32)  # [H, I]
        wu = n2[l][:, None] * np.asarray(inputs["w_up"][l], np.float32)
        wd = np.asarray(inputs["w_down"][l], np.float32)                   # [I, H]
        gpart = wg.reshape(KT, 128, NC2, 2, 128).transpose(2, 1, 0, 3, 4) \
            .reshape(NC2, 128, 2048)
        upart = wu.reshape(KT, 128, NC2, 2, 128).transpose(2, 1, 0, 3, 4) \
            .reshape(NC2, 128, 2048)
        dpart = wd.reshape(NC2, 2, 128, KT, 128).transpose(0, 2, 1, 3, 4) \
            .reshape(NC2, 128, 2048)
        common[f"wm{l}"] = bf(np.concatenate([gpart, upart, dpart], axis=2))

    in_maps = []
    for core in range(NCORE):
        b, qc = core // 4, core % 4
        pos = np.arange(T, dtype=np.float64) + qc * T
        freqs = np.outer(pos, inv_freq)
        emb = np.concatenate([freqs, freqs], axis=-1)
        cosT = np.cos(emb).T.astype(np.float32)     # [64, T]
        sinT = np.sin(emb).T.astype(np.float32)
        qpos = (np.arange(T) + qc * T)
        kvpos = (np.arange(8)[None, :] * 128 + np.arange(128)[:, None])  # [128, 8]
        mask = (kvpos[:, :, None] <= qpos[None, None, :]).astype(BF16NP)
        x0T = embed[ids[b, qc * T:(qc + 1) * T]].T.astype(np.float32)    # [H, T]
        x0T = np.ascontiguousarray(x0T.reshape(KT, 128, T).transpose(1, 0, 2))
        vbase = qc * VSH
        embs = (nf[:, None] * embed[vbase:vbase + VSH].T)                # [H, VSH]
        embs = bf(embs.reshape(KT, 128, NVC, VC).transpose(2, 1, 0, 3))
        m = dict(common)
        m.update({
            "x0T": x0T,
            "cos2": np.ascontiguousarray(np.tile(cosT, (2, 1))),
            "sin2": np.ascontiguousarray(np.tile(sinT, (2, 1))),
            "mask": np.ascontiguousarray(mask),
            "embT": embs,
        })
        in_maps.append(m)
    return in_maps


def _get_program():
    if "prog" not in _CACHE:
        _CACHE["prog"] = build_program()
    return _CACHE["prog"]


def run(inputs, debug_layers=False, trace=False):
    nc = _get_program()
    in_maps = _host_prep(inputs)
    res = run_bass_kernel_spmd(nc, in_maps, core_ids=list(range(NCORE)), trace=trace)
    out = np.zeros((B, S, V), np.float32)
    for b in range(B):
        for qc in range(4):
            lt = res.results[4 * b + qc]["logits_t"]     # [NVC, 128, 8, VC]
            shard = lt.transpose(2, 1, 0, 3).reshape(S, VSH)
            out[b, :, qc * VSH:(qc + 1) * VSH] = shard
    return out, res


def kernel(**inputs) -> np.ndarray:
    out, _ = run(inputs)
    return out


# revision 12
# speedup vs baseline: 1.5110x; 1.0696x over previous
"""Trainium2 Bass kernel for a 4-layer GQA transformer LM (nn_CustomLLM_35278861369705).

Sharding: sequence-parallel across 8 cores - 2 batch groups x 4 sequence chunks
of 256 tokens. Activations kept transposed [feature, token] on device.
All matmul operands are bf16 (fp32 PSUM accumulation, fp32 residual stream).
Weights are pre-laid out on the host in tile-ready form so each weight tensor
loads with a single large fully-contiguous DMA; MLP weights stream in 16
chunks of 1.5MB (gate+up+down packed) with triple buffering.
Per layer: RMSNorm (ones-matmul partition reduction), fused-rope QKV,
split K / V group AllGathers (K gathered while Q/V project), masked full-kv
attention, SwiGLU MLP with PSUM-resident down-proj accumulators.
Final AllGather of hidden states + vocab-sharded tied LM head with SBUF-staged
logits written back in 2MB tiles.
"""
import numpy as np
import ml_dtypes

import concourse.bass as bass
import concourse.mybir as mybir
import concourse.tile as tile
from concourse import bacc
from concourse.bass_utils import run_bass_kernel_spmd

V, H, NH, KVH, I, L, S, B = 32000, 1024, 16, 4, 4096, 4, 1024, 2
HD = 64
THETA = 10000.0
EPS = 1e-5
T = 256            # tokens per core
NCORE = 8
GROUPS = [[0, 1, 2, 3], [4, 5, 6, 7]]
VSH = V // 4       # vocab shard per core (within its 4-core group)
KT = H // 128      # 8
NC2 = 16           # MLP I-chunks per layer (256 feats each)
NVC = 16           # vocab chunks per core
VC = VSH // NVC    # 500

F32 = mybir.dt.float32
BF16 = mybir.dt.bfloat16
AF = mybir.ActivationFunctionType
BF16NP = ml_dtypes.bfloat16

# MLP chunk free-dim offsets (elements within the flat [128, 6144] tile)
GOFF = 0           # gate region:  (kt, fi) -> (kt*2+fi)*128
UOFF = 2048        # up region:    (kt, fi) -> UOFF + (kt*2+fi)*128
DOFF = 4096        # down region:  (ci, m)  -> DOFF + (ci*8+m)*128

_CACHE = {}


def build_program():
    nc = bacc.Bacc("TRN2", target_bir_lowering=False, debug=False,
                   num_devices=NCORE)

    # ---------------- I/O ----------------
    x0T = nc.dram_tensor("x0T", [128, KT, T], F32, kind="ExternalInput").ap()
    cos2 = nc.dram_tensor("cos2", [128, T], F32, kind="ExternalInput").ap()
    sin2 = nc.dram_tensor("sin2", [128, T], F32, kind="ExternalInput").ap()
    ropeR = nc.dram_tensor("ropeR", [128, 128], BF16, kind="ExternalInput").ap()
    ones_in = nc.dram_tensor("ones_in", [128, 128], BF16, kind="ExternalInput").ap()
    mask_in = nc.dram_tensor("mask", [128, 8, T], BF16, kind="ExternalInput").ap()
    embT = nc.dram_tensor("embT", [NVC, 128, KT, VC], BF16, kind="ExternalInput").ap()
    wq_d, wk_d, wv_d, wo_d, wm_d = [], [], [], [], []
    for l in range(L):
        wq_d.append(nc.dram_tensor(f"wq{l}", [128, KT, KT, 128], BF16,
                                   kind="ExternalInput").ap())   # [p, m, kt, of]
        wk_d.append(nc.dram_tensor(f"wk{l}", [128, 2, KT, 128], BF16,
                                   kind="ExternalInput").ap())   # [p, m, kt, of]
        wv_d.append(nc.dram_tensor(f"wv{l}", [128, KT, 256], BF16,
                                   kind="ExternalInput").ap())   # [p, kt, of]
        wo_d.append(nc.dram_tensor(f"wo{l}", [128, KT, KT, 128], BF16,
                                   kind="ExternalInput").ap())   # [p, j, m, of]
        wm_d.append(nc.dram_tensor(f"wm{l}", [NC2, 128, 6144], BF16,
                                   kind="ExternalInput").ap())   # packed g/u/d
    logits_t = nc.dram_tensor("logits_t", [NVC, 128, 8, VC], F32,
                              kind="ExternalOutput").ap()

    with tile.TileContext(nc) as tc:
        with (
            tc.tile_pool(name="const", bufs=1) as cpool,
            tc.tile_pool(name="xres", bufs=1) as xpool,
            tc.tile_pool(name="hnorm", bufs=2) as hpool,
            tc.tile_pool(name="sqp", bufs=2) as sqpool,
            tc.tile_pool(name="tmps", bufs=3) as tpool,
            tc.tile_pool(name="dram", bufs=2, space="DRAM") as dpool,
        ):
            # ---- residual stream + first-needed constants first ----
            xT = xpool.tile([128, KT, T], F32, tag="xT")
            nc.sync.dma_start(xT[:], x0T[:])
            ones_sb = cpool.tile([128, 128], BF16, tag="ones")
            nc.sync.dma_start(ones_sb[:], ones_in[:])
            cos_sb = cpool.tile([128, T], F32, tag="cos")
            sin_sb = cpool.tile([128, T], F32, tag="sin")
            nc.sync.dma_start(cos_sb[:], cos2[:])
            nc.sync.dma_start(sin_sb[:], sin2[:])
            ropeR_sb = cpool.tile([128, 128], BF16, tag="ropeR")
            nc.sync.dma_start(ropeR_sb[:], ropeR[:])
            mask_sb = cpool.tile([128, 8, T], BF16, tag="mask")
            nc.sync.dma_start(mask_sb[:], mask_in[:])

            def rmsnorm(src):
                """src: [128, KT, T] f32 -> hT [128, KT, T] bf16 (norm weights
                are folded into the following matmul weights on host).
                Per-kt pipelined so PE sum-matmuls overlap the DVE squares."""
                sq = sqpool.tile([128, KT, T], BF16, tag="sq")
                with tc.tile_pool(name="psnorm", bufs=1, space="PSUM") as pp:
                    ps = pp.tile([128, T], F32, tag="ps_norm")
                    for kt in range(KT):
                        nc.vector.tensor_mul(out=sq[:, kt], in0=src[:, kt],
                                             in1=src[:, kt])
                        nc.tensor.matmul(ps[:], ones_sb[:], sq[:, kt],
                                         start=(kt == 0), stop=(kt == KT - 1))
                    ms = tpool.tile([128, T], F32, tag="ms")
                    nc.scalar.activation(ms[:], ps[:], AF.Copy, bias=EPS, scale=1.0 / H)
                rcp = tpool.tile([128, T], F32, tag="rcp")
                nc.vector.reciprocal_approx_fast(out=rcp[:], in_=ms[:])
                inv = tpool.tile([128, T], F32, tag="inv")
                nc.scalar.activation(inv[:], rcp[:], AF.Sqrt)
                hT = hpool.tile([128, KT, T], BF16, tag="h")
                for kt in range(KT):
                    nc.vector.tensor_mul(out=hT[:, kt], in0=src[:, kt],
                                         in1=inv[:])
                return hT

            # =================== layers ===================
            layer_scope = (
                tc.tile_pool(name="wqp", bufs=1),
                tc.tile_pool(name="wkp", bufs=2),
                tc.tile_pool(name="wvp", bufs=2),
                tc.tile_pool(name="wop", bufs=1),
                tc.tile_pool(name="wmp", bufs=3),
                tc.tile_pool(name="acts", bufs=1),
            )
            wqpool, wkpool, wvpool, wopool, wmpool, apool = \
                [p.__enter__() for p in layer_scope]
            for l in range(L):
                with nc.named_scope(f"layer{l}_qkv"):
                    hT = rmsnorm(xT)
                    # single big weight DMAs (fully contiguous layouts)
                    wqt = wqpool.tile([128, KT, KT, 128], BF16, tag="w_q")
                    nc.sync.dma_start(wqt[:], wq_d[l][:])
                    wkt = wkpool.tile([128, 2, KT, 128], BF16, tag="w_k")
                    nc.sync.dma_start(wkt[:], wk_d[l][:])
                    wvt = wvpool.tile([128, KT, 256], BF16, tag="w_v")
                    nc.sync.dma_start(wvt[:], wv_d[l][:])

                    qT = apool.tile([128, KT, T], BF16, tag="qT")
                    kT_loc = apool.tile([128, 2, T], BF16, tag="kT_loc")
                    v_loc = apool.tile([128, 2, T], BF16, tag="v_loc")

                    with tc.tile_pool(name="psqkv", bufs=2, space="PSUM") as pq:
                        def proj_rope(wt_sl, out_sl):
                            """one 128-feature projection slice + rope -> out_sl."""
                            ps = pq.tile([128, T], F32, tag="ps_qkv")
                            for kt in range(KT):
                                nc.tensor.matmul(ps[:], wt_sl[:, kt], hT[:, kt],
                                                 start=(kt == 0), stop=(kt == KT - 1))
                            raw = tpool.tile([128, T], BF16, tag="qraw")
                            nc.vector.tensor_copy(out=raw[:], in_=ps[:])
                            rot = pq.tile([128, T], F32, tag="ps_rot")
                            nc.tensor.matmul(rot[:], ropeR_sb[:], raw[:],
                                             start=True, stop=True)
                            tcs = tpool.tile([128, T], F32, tag="tcos")
                            nc.vector.tensor_mul(out=tcs[:], in0=ps[:], in1=cos_sb[:])
                            tsn = tpool.tile([128, T], F32, tag="tsin")
                            nc.vector.tensor_mul(out=tsn[:], in0=rot[:], in1=sin_sb[:])
                            nc.vector.tensor_add(out=out_sl, in0=tcs[:], in1=tsn[:])

                        # K first so its AllGather overlaps Q/V compute
                        for m in range(2):
                            proj_rope(wkt[:, m], kT_loc[:, m, :])
                        cck_in = dpool.tile([2 * 128, T], BF16, tag="cck_in")
                        nc.sync.dma_start(
                            cck_in.rearrange("(s p) t -> p s t", p=128), kT_loc[:])
                        cck_out = dpool.tile([4 * 2 * 128, T], BF16, tag="cck_out")
                        nc.gpsimd.collective_compute(
                            "AllGather", mybir.AluOpType.bypass,
                            ins=[cck_in.opt()], outs=[cck_out.opt()],
                            replica_groups=GROUPS)
                        cck_r = cck_out.rearrange("(c s p) t -> p c s t", c=4, s=2)

                        for m in range(KT):
                            proj_rope(wqt[:, m], qT[:, m, :])

                        # V in natural [token, feature] layout
                        for tt in range(2):
                            psv = pq.tile([128, T], F32, tag="ps_qkv")
                            for kt in range(KT):
                                nc.tensor.matmul(psv[:], hT[:, kt, tt * 128:(tt + 1) * 128],
                                                 wvt[:, kt],
                                                 start=(kt == 0), stop=(kt == KT - 1))
                            nc.vector.tensor_copy(out=v_loc[:, tt, :], in_=psv[:])
                        ccv_in = dpool.tile([2 * 128, T], BF16, tag="ccv_in")
                        nc.sync.dma_start(
                            ccv_in.rearrange("(s p) t -> p s t", p=128), v_loc[:])
                        ccv_out = dpool.tile([4 * 2 * 128, T], BF16, tag="ccv_out")
                        nc.gpsimd.collective_compute(
                            "AllGather", mybir.AluOpType.bypass,
                            ins=[ccv_in.opt()], outs=[ccv_out.opt()],
                            replica_groups=GROUPS)
                        ccv_r = ccv_out.rearrange("(c s p) t -> p c s t", c=4, s=2)

                with nc.named_scope(f"layer{l}_attn"):
                    # o-proj weights: issue DMA early so it overlaps attention
                    wot = wopool.tile([128, KT, KT, 128], BF16, tag="w_o")
                    nc.sync.dma_start(wot[:], wo_d[l][:])

                    kg = []
                    for g in range(KVH):
                        # duplicate k rows into both partition halves so matmuls
                        # with q heads at base 0 or 64 both have matching bases
                        kgt = apool.tile([128, 4, T], BF16, tag=f"kg{g}")
                        src = cck_r[64 * (g % 2):64 * (g % 2) + 64, :, g // 2, :]
                        nc.sync.dma_start(kgt[0:64], src)
                        nc.sync.dma_start(kgt[64:128], src)
                        kg.append(kgt)
                    # all gathered V in one contiguous SBUF tile; attention
                    # slices it directly as the stationary operand
                    vall = apool.tile([128, 4, 2, T], BF16, tag="vall")
                    nc.sync.dma_start(vall[:], ccv_r[:])

                    oT = apool.tile([128, KT, T], BF16, tag="oT")
                    with (
                        tc.tile_pool(name="psatt", bufs=2, space="PSUM") as pa,
                        tc.tile_pool(name="pssum", bufs=2, space="PSUM") as pas,
                        tc.tile_pool(name="pexp", bufs=5) as epool,
                    ):
                        for h in range(NH):
                            g = h // 4
                            base = 64 * (h % 2)
                            q_sl = qT[base:base + 64, h // 2, :]
                            pjs = []
                            for c2 in range(2):
                                # scores for a PAIR of kv chunks -> one exp+mask
                                ps_s = pa.tile([128, 4, T], F32, tag="ps_s")
                                for jj in range(4):
                                    c = 2 * c2 + jj // 2
                                    mt = jj % 2
                                    nc.tensor.matmul(
                                        ps_s[:, jj, :],
                                        kg[g][base:base + 64, c, mt * 128:(mt + 1) * 128],
                                        q_sl, start=True, stop=True)
                                e1 = epool.tile([128, 4, T], BF16, tag="e1")
                                nc.scalar.activation(e1[:], ps_s[:], AF.Exp, scale=0.125)
                                pj = epool.tile([128, 4, T], BF16, tag="pj")
                                nc.vector.tensor_mul(out=pj[:], in0=e1[:],
                                                     in1=mask_sb[:, 4 * c2:4 * c2 + 4, :])
                                pjs.append(pj)
                            ps_sum = pas.tile([128, T], F32, tag="ps_sum")
                            ps_o = pas.tile([64, T], F32, tag="ps_o")
                            for j in range(8):
                                c = j // 2
                                tt = j % 2
                                pj_sl = pjs[c // 2][:, (c % 2) * 2 + tt, :]
                                nc.tensor.matmul(ps_sum[:], ones_sb[:], pj_sl,
                                                 start=(j == 0), stop=(j == 7))
                                nc.tensor.matmul(
                                    ps_o[:],
                                    vall[:, c, tt, g * HD:(g + 1) * HD],
                                    pj_sl,
                                    start=(j == 0), stop=(j == 7))
                            invb = epool.tile([128, T], F32, tag="invb")
                            nc.vector.reciprocal_approx_fast(out=invb[:], in_=ps_sum[:])
                            nc.vector.tensor_mul(out=oT[base:base + 64, h // 2, :],
                                                 in0=ps_o[:], in1=invb[0:64, :])

                    # ---- o-projection + residual ----
                    with tc.tile_pool(name="psoproj", bufs=2, space="PSUM") as po:
                        for m in range(KT):
                            ps = po.tile([128, T], F32, tag="ps_op")
                            for j in range(KT):
                                nc.tensor.matmul(ps[:], wot[:, j, m], oT[:, j, :],
                                                 start=(j == 0), stop=(j == KT - 1))
                            nc.vector.tensor_add(out=xT[:, m, :], in0=xT[:, m, :],
                                                 in1=ps[:])

                with nc.named_scope(f"layer{l}_mlp"):
                    h2T = rmsnorm(xT)
                    with (
                        tc.tile_pool(name="psmlpd", bufs=1, space="PSUM") as pmd,
                        tc.tile_pool(name="psmlp", bufs=2, space="PSUM") as pm,
                    ):
                        ps_d = [pmd.tile([128, 2, T], F32, tag=f"ps_d{i}", name=f"ps_d{i}")
                                for i in range(4)]
                        for c in range(NC2):
                            wmt = wmpool.tile([128, 6144], BF16, tag="wmlp")
                            nc.sync.dma_start(wmt[:], wm_d[l][c])
                            for fi in range(2):
                                ps_g = pm.tile([128, T], F32, tag="ps_g")
                                for kt in range(KT):
                                    off = GOFF + (kt * 2 + fi) * 128
                                    nc.tensor.matmul(ps_g[:], wmt[:, off:off + 128],
                                                     h2T[:, kt],
                                                     start=(kt == 0), stop=(kt == KT - 1))
                                ps_u = pm.tile([128, T], F32, tag="ps_u")
                                for kt in range(KT):
                                    off = UOFF + (kt * 2 + fi) * 128
                                    nc.tensor.matmul(ps_u[:], wmt[:, off:off + 128],
                                                     h2T[:, kt],
                                                     start=(kt == 0), stop=(kt == KT - 1))
                                silu = tpool.tile([128, T], F32, tag="silu")
                                nc.scalar.activation(silu[:], ps_g[:], AF.Silu)
                                gu = tpool.tile([128, T], BF16, tag="gu")
                                nc.vector.tensor_mul(out=gu[:], in0=silu[:], in1=ps_u[:])
                                first = (c == 0 and fi == 0)
                                last = (c == NC2 - 1 and fi == 1)
                                for m in range(KT):
                                    off = DOFF + (fi * 8 + m) * 128
                                    # start=True clears the WHOLE bank's has_written,
                                    # so only the first matmul touching each bank may
                                    # set it; the odd slice's first write then stores
                                    # (has_written=0) and later writes accumulate.
                                    nc.tensor.matmul(ps_d[m // 2][:, m % 2, :],
                                                     wmt[:, off:off + 128], gu[:],
                                                     start=(first and m % 2 == 0),
                                                     stop=last,
                                                     skip_group_check=True)
                        for m in range(KT):
                            nc.vector.tensor_add(out=xT[:, m, :], in0=xT[:, m, :],
                                                 in1=ps_d[m // 2][:, m % 2, :])

            for p in reversed(layer_scope):
                p.__exit__(None, None, None)

            # =================== LM head ===================
            with nc.named_scope("lm_head"):
                hfT = rmsnorm(xT)
                cc2_in = dpool.tile([H, T], BF16, tag="cc2_in")
                nc.scalar.dma_start(cc2_in.rearrange("(kt p) t -> p kt t", p=128),
                                    hfT[:])
                cc2_out = dpool.tile([4 * H, T], BF16, tag="cc2_out")
                nc.gpsimd.collective_compute(
                    "AllGather", mybir.AluOpType.bypass,
                    ins=[cc2_in.opt()], outs=[cc2_out.opt()],
                    replica_groups=GROUPS)
                cc2_r = cc2_out.rearrange("(c kt p) t -> p c kt t", c=4, kt=KT)

                with (
                    tc.tile_pool(name="hall", bufs=1) as hallp,
                    tc.tile_pool(name="embp", bufs=3) as embp,
                    tc.tile_pool(name="lsbp", bufs=2) as lsbp,
                    tc.tile_pool(name="pslm", bufs=4, space="PSUM") as plm,
                ):
                    ha = []
                    for m8 in range(8):
                        hat = hallp.tile([128, KT, 128], BF16, tag=f"ha{m8}")
                        nc.scalar.dma_start(
                            hat[:],
                            cc2_r[:, m8 // 2, :, 128 * (m8 % 2):128 * (m8 % 2) + 128])
                        ha.append(hat)
                    for vc in range(NVC):
                        et = embp.tile([128, KT, VC], BF16, tag="emb")
                        nc.sync.dma_start(et[:], embT[vc])
                        lsb = lsbp.tile([128, 8, VC], F32, tag="lsb")
                        for m8 in range(8):
                            ps = plm.tile([128, VC], F32, tag="ps_lm")
                            for kt in range(KT):
                                nc.tensor.matmul(ps[:], ha[m8][:, kt], et[:, kt],
                                                 start=(kt == 0), stop=(kt == KT - 1))
                            nc.vector.tensor_copy(out=lsb[:, m8, :], in_=ps[:])
                        nc.sync.dma_start(logits_t[vc], lsb[:])

    nc.finalize()
    return nc


# ---------------- host side ----------------

def _host_prep(inputs):
    """Build per-core input maps from full inputs (tile-ready bf16 layouts)."""
    ids = np.asarray(inputs["input_ids"])
    embed = np.asarray(inputs["embed"], dtype=np.float32)
    n1 = np.asarray(inputs["norm1_w"], dtype=np.float32)
    n2 = np.asarray(inputs["norm2_w"], dtype=np.float32)
    nf = np.asarray(inputs["final_norm_w"], dtype=np.float32)

    inv_freq = 1.0 / (THETA ** (np.arange(0, HD, 2, dtype=np.float64) / HD))
    R64 = np.zeros((HD, HD), np.float32)
    for i in range(32):
        R64[i, i + 32] = -1.0
        R64[i + 32, i] = 1.0
    Rblk = np.zeros((128, 128), np.float32)
    Rblk[:64, :64] = R64
    Rblk[64:, 64:] = R64
    ropeR = np.ascontiguousarray(Rblk.T).astype(BF16NP)
    ones128 = np.ones((128, 128), BF16NP)

    def bf(x):
        return np.ascontiguousarray(x).astype(BF16NP)

    common = {"ropeR": ropeR, "ones_in": ones128}
    for l in range(L):
        wq = n1[l][:, None] * np.asarray(inputs["wq"][l], np.float32)    # [H, H]
        common[f"wq{l}"] = bf(wq.reshape(KT, 128, KT, 128).transpose(1, 2, 0, 3))
        wk = n1[l][:, None] * np.asarray(inputs["wk"][l], np.float32)    # [H, 256]
        common[f"wk{l}"] = bf(wk.reshape(KT, 128, 2, 128).transpose(1, 2, 0, 3))
        wv = n1[l][:, None] * np.asarray(inputs["wv"][l], np.float32)    # [H, 256]
        common[f"wv{l}"] = bf(wv.reshape(KT, 128, 256).transpose(1, 0, 2))
        wo = np.asarray(inputs["wo"][l], np.float32)                     # [H, H]
        # in-feature index = 64*(2j+s)+d -> partition p = 64*s+d, free j
        common[f"wo{l}"] = bf(wo.reshape(KT, 2, 64, KT, 128).transpose(1, 2, 0, 3, 4)
                              .reshape(128, KT, KT, 128))
        wg = n2[l][:, None] * np.asarray(inputs["w_gate"][l], np.float32)  # [H, I]
        wu = n2[l][:, None] * np.asarray(inputs["w_up"][l], np.float32)
        wd = np.asarray(inputs["w_down"][l], np.float32)                   # [I, H]
        gpart = wg.reshape(KT, 128, NC2, 2, 128).transpose(2, 1, 0, 3, 4) \
            .reshape(NC2, 128, 2048)
        upart = wu.reshape(KT, 128, NC2, 2, 128).transpose(2, 1, 0, 3, 4) \
            .reshape(NC2, 128, 2048)
        dpart = wd.reshape(NC2, 2, 128, KT, 128).transpose(0, 2, 1, 3, 4) \
            .reshape(NC2, 128, 2048)
        common[f"wm{l}"] = bf(np.concatenate([gpart, upart, dpart], axis=2))

    in_maps = []
    for core in range(NCORE):
        b, qc = core // 4, core % 4
        pos = np.arange(T, dtype=np.float64) + qc * T
        freqs = np.outer(pos, inv_freq)
        emb = np.concatenate([freqs, freqs], axis=-1)
        cosT = np.cos(emb).T.astype(np.float32)     # [64, T]
        sinT = np.sin(emb).T.astype(np.float32)
        qpos = (np.arange(T) + qc * T)
        kvpos = (np.arange(8)[None, :] * 128 + np.arange(128)[:, None])  # [128, 8]
        mask = (kvpos[:, :, None] <= qpos[None, None, :]).astype(BF16NP)
        x0T = embed[ids[b, qc * T:(qc + 1) * T]].T.astype(np.float32)    # [H, T]
        x0T = np.ascontiguousarray(x0T.reshape(KT, 128, T).transpose(1, 0, 2))
        vbase = qc * VSH
        embs = (nf[:, None] * embed[vbase:vbase + VSH].T)                # [H, VSH]
        embs = bf(embs.reshape(KT, 128, NVC, VC).transpose(2, 1, 0, 3))
        m = dict(common)
        m.update({
            "x0T": x0T,
            "cos2": np.ascontiguousarray(np.tile(cosT, (2, 1))),
            "sin2": np.ascontiguousarray(np.tile(sinT, (2, 1))),
            "mask": np.ascontiguousarray(mask),
            "embT": embs,
        })
        in_maps.append(m)
    return in_maps


def _get_program():
    if "prog" not in _CACHE:
        _CACHE["prog"] = build_program()
    return _CACHE["prog"]


def run(inputs, debug_layers=False, trace=False):
    nc = _get_program()
    in_maps = _host_prep(inputs)
    res = run_bass_kernel_spmd(nc, in_maps, core_ids=list(range(NCORE)), trace=trace)
    out = np.zeros((B, S, V), np.float32)
    for b in range(B):
        for qc in range(4):
            lt = res.results[4 * b + qc]["logits_t"]     # [NVC, 128, 8, VC]
            shard = lt.transpose(2, 1, 0, 3).reshape(S, VSH)
            out[b, :, qc * VSH:(qc + 1) * VSH] = shard
    return out, res


def kernel(**inputs) -> np.ndarray:
    out, _ = run(inputs)
    return out


# revision 14
# speedup vs baseline: 1.6468x; 1.0899x over previous
"""Trainium2 Bass kernel for a 4-layer GQA transformer LM (nn_CustomLLM_35278861369705).

Sharding: sequence-parallel across 8 cores - 2 batch groups x 4 sequence chunks
of 256 tokens. Activations kept transposed [feature, token] on device.
All matmul operands are bf16 (fp32 PSUM accumulation, fp32 residual stream).
Weights are pre-laid out on the host in tile-ready form so each weight tensor
loads with a single large fully-contiguous DMA; MLP weights stream in 16
chunks of 1.5MB (gate+up+down packed) with triple buffering.
Per layer: RMSNorm (ones-matmul partition reduction), fused-rope QKV,
split K / V group AllGathers (K gathered while Q/V project), masked full-kv
attention, SwiGLU MLP with PSUM-resident down-proj accumulators.
Final AllGather of hidden states + vocab-sharded tied LM head with SBUF-staged
logits written back in 2MB tiles.
"""
import numpy as np
import ml_dtypes

import concourse.bass as bass
import concourse.mybir as mybir
import concourse.tile as tile
from concourse import bacc
from concourse.bass_utils import run_bass_kernel_spmd

V, H, NH, KVH, I, L, S, B = 32000, 1024, 16, 4, 4096, 4, 1024, 2
HD = 64
THETA = 10000.0
EPS = 1e-5
T = 256            # tokens per core
NCORE = 8
GROUPS = [[0, 1, 2, 3], [4, 5, 6, 7]]
VSH = V // 4       # vocab shard per core (within its 4-core group)
KT = H // 128      # 8
NC2 = 16           # MLP I-chunks per layer (256 feats each)
NVC = 16           # vocab chunks per core
VC = VSH // NVC    # 500

F32 = mybir.dt.float32
BF16 = mybir.dt.bfloat16
AF = mybir.ActivationFunctionType
BF16NP = ml_dtypes.bfloat16

# MLP chunk free-dim offsets (elements within the flat [128, 6144] tile)
GOFF = 0           # gate region:  (kt, fi) -> (kt*2+fi)*128
UOFF = 2048        # up region:    (kt, fi) -> UOFF + (kt*2+fi)*128
DOFF = 4096        # down region:  (ci, m)  -> DOFF + (ci*8+m)*128

_CACHE = {}


def build_program():
    nc = bacc.Bacc("TRN2", target_bir_lowering=False, debug=False,
                   num_devices=NCORE)

    # ---------------- I/O ----------------
    x0T = nc.dram_tensor("x0T", [128, KT, T], F32, kind="ExternalInput").ap()
    cos2 = nc.dram_tensor("cos2", [128, T], F32, kind="ExternalInput").ap()
    sin2 = nc.dram_tensor("sin2", [128, T], F32, kind="ExternalInput").ap()
    ropeR = nc.dram_tensor("ropeR", [128, 128], BF16, kind="ExternalInput").ap()
    ones_in = nc.dram_tensor("ones_in", [128, 128], BF16, kind="ExternalInput").ap()
    mask_in = nc.dram_tensor("mask", [128, 8, T], BF16, kind="ExternalInput").ap()
    embT = nc.dram_tensor("embT", [NVC, 128, KT, VC], BF16, kind="ExternalInput").ap()
    wq_d, wk_d, wv_d, wo_d, wm_d = [], [], [], [], []
    for l in range(L):
        wq_d.append(nc.dram_tensor(f"wq{l}", [128, KT, KT, 128], BF16,
                                   kind="ExternalInput").ap())   # [p, m, kt, of]
        wk_d.append(nc.dram_tensor(f"wk{l}", [128, 2, KT, 128], BF16,
                                   kind="ExternalInput").ap())   # [p, m, kt, of]
        wv_d.append(nc.dram_tensor(f"wv{l}", [128, KT, 256], BF16,
                                   kind="ExternalInput").ap())   # [p, kt, of]
        wo_d.append(nc.dram_tensor(f"wo{l}", [128, KT, KT, 128], BF16,
                                   kind="ExternalInput").ap())   # [p, j, m, of]
        wm_d.append(nc.dram_tensor(f"wm{l}", [NC2, 128, 6144], BF16,
                                   kind="ExternalInput").ap())   # packed g/u/d
    logits_t = nc.dram_tensor("logits_t", [NVC, 128, 8, VC], F32,
                              kind="ExternalOutput").ap()

    with tile.TileContext(nc) as tc:
        with (
            tc.tile_pool(name="const", bufs=1) as cpool,
            tc.tile_pool(name="xres", bufs=1) as xpool,
            tc.tile_pool(name="hnorm", bufs=2) as hpool,
            tc.tile_pool(name="sqp", bufs=2) as sqpool,
            tc.tile_pool(name="tmps", bufs=3) as tpool,
            tc.tile_pool(name="dram", bufs=2, space="DRAM") as dpool,
        ):
            # ---- residual stream + first-needed constants first ----
            xT = xpool.tile([128, KT, T], F32, tag="xT")
            nc.sync.dma_start(xT[:], x0T[:])
            ones_sb = cpool.tile([128, 128], BF16, tag="ones")
            nc.sync.dma_start(ones_sb[:], ones_in[:])
            cos_sb = cpool.tile([128, T], F32, tag="cos")
            sin_sb = cpool.tile([128, T], F32, tag="sin")
            nc.sync.dma_start(cos_sb[:], cos2[:])
            nc.sync.dma_start(sin_sb[:], sin2[:])
            ropeR_sb = cpool.tile([128, 128], BF16, tag="ropeR")
            nc.sync.dma_start(ropeR_sb[:], ropeR[:])
            mask_sb = cpool.tile([128, 8, T], BF16, tag="mask")
            nc.sync.dma_start(mask_sb[:], mask_in[:])

            def rmsnorm(src):
                """src: [128, KT, T] f32 -> hT [128, KT, T] bf16 (norm weights
                are folded into the following matmul weights on host).
                Per-kt pipelined so PE sum-matmuls overlap the DVE squares."""
                sq = sqpool.tile([128, KT, T], BF16, tag="sq")
                with tc.tile_pool(name="psnorm", bufs=1, space="PSUM") as pp:
                    ps = pp.tile([128, T], F32, tag="ps_norm")
                    for kt in range(KT):
                        nc.vector.tensor_mul(out=sq[:, kt], in0=src[:, kt],
                                             in1=src[:, kt])
                        nc.tensor.matmul(ps[:], ones_sb[:], sq[:, kt],
                                         start=(kt == 0), stop=(kt == KT - 1))
                    ms = tpool.tile([128, T], F32, tag="ms")
                    nc.scalar.activation(ms[:], ps[:], AF.Copy, bias=EPS, scale=1.0 / H)
                rcp = tpool.tile([128, T], F32, tag="rcp")
                nc.vector.reciprocal_approx_fast(out=rcp[:], in_=ms[:])
                inv = tpool.tile([128, T], F32, tag="inv")
                nc.scalar.activation(inv[:], rcp[:], AF.Sqrt)
                hT = hpool.tile([128, KT, T], BF16, tag="h")
                for kt in range(KT):
                    nc.vector.tensor_mul(out=hT[:, kt], in0=src[:, kt],
                                         in1=inv[:])
                return hT

            # =================== layers ===================
            layer_scope = (
                tc.tile_pool(name="wqp", bufs=1),
                tc.tile_pool(name="wkp", bufs=2),
                tc.tile_pool(name="wvp", bufs=2),
                tc.tile_pool(name="wop", bufs=1),
                tc.tile_pool(name="wmp", bufs=3),
                tc.tile_pool(name="acts", bufs=1),
            )
            wqpool, wkpool, wvpool, wopool, wmpool, apool = \
                [p.__enter__() for p in layer_scope]
            for l in range(L):
                with nc.named_scope(f"layer{l}_qkv"):
                    hT = rmsnorm(xT)
                    # single big weight DMAs (fully contiguous layouts)
                    wqt = wqpool.tile([128, KT, KT, 128], BF16, tag="w_q")
                    nc.sync.dma_start(wqt[:], wq_d[l][:])
                    wkt = wkpool.tile([128, 2, KT, 128], BF16, tag="w_k")
                    nc.sync.dma_start(wkt[:], wk_d[l][:])
                    wvt = wvpool.tile([128, KT, 256], BF16, tag="w_v")
                    nc.sync.dma_start(wvt[:], wv_d[l][:])

                    qT = apool.tile([128, KT, T], BF16, tag="qT")
                    kT_loc = apool.tile([128, 2, T], BF16, tag="kT_loc")
                    v_loc = apool.tile([128, 2, T], BF16, tag="v_loc")

                    with tc.tile_pool(name="psqkv", bufs=2, space="PSUM") as pq:
                        def proj_rope(wt_sl, out_sl):
                            """one 128-feature projection slice + rope -> out_sl."""
                            ps = pq.tile([128, T], F32, tag="ps_qkv")
                            for kt in range(KT):
                                nc.tensor.matmul(ps[:], wt_sl[:, kt], hT[:, kt],
                                                 start=(kt == 0), stop=(kt == KT - 1))
                            raw = tpool.tile([128, T], BF16, tag="qraw")
                            nc.vector.tensor_copy(out=raw[:], in_=ps[:])
                            rot = pq.tile([128, T], F32, tag="ps_rot")
                            nc.tensor.matmul(rot[:], ropeR_sb[:], raw[:],
                                             start=True, stop=True)
                            tcs = tpool.tile([128, T], F32, tag="tcos")
                            nc.vector.tensor_mul(out=tcs[:], in0=ps[:], in1=cos_sb[:])
                            tsn = tpool.tile([128, T], F32, tag="tsin")
                            nc.vector.tensor_mul(out=tsn[:], in0=rot[:], in1=sin_sb[:])
                            nc.vector.tensor_add(out=out_sl, in0=tcs[:], in1=tsn[:])

                        # K first so its AllGather overlaps Q/V compute
                        for m in range(2):
                            proj_rope(wkt[:, m], kT_loc[:, m, :])
                        cck_in = dpool.tile([2 * 128, T], BF16, tag="cck_in")
                        nc.sync.dma_start(
                            cck_in.rearrange("(s p) t -> p s t", p=128), kT_loc[:])
                        cck_out = dpool.tile([4 * 2 * 128, T], BF16, tag="cck_out")
                        nc.gpsimd.collective_compute(
                            "AllGather", mybir.AluOpType.bypass,
                            ins=[cck_in.opt()], outs=[cck_out.opt()],
                            replica_groups=GROUPS)
                        cck_r = cck_out.rearrange("(c s p) t -> p c s t", c=4, s=2)

                        for m in range(KT):
                            proj_rope(wqt[:, m], qT[:, m, :])

                        # V in natural [token, feature] layout
                        for tt in range(2):
                            psv = pq.tile([128, T], F32, tag="ps_qkv")
                            for kt in range(KT):
                                nc.tensor.matmul(psv[:], hT[:, kt, tt * 128:(tt + 1) * 128],
                                                 wvt[:, kt],
                                                 start=(kt == 0), stop=(kt == KT - 1))
                            nc.vector.tensor_copy(out=v_loc[:, tt, :], in_=psv[:])
                        ccv_in = dpool.tile([2 * 128, T], BF16, tag="ccv_in")
                        nc.sync.dma_start(
                            ccv_in.rearrange("(s p) t -> p s t", p=128), v_loc[:])
                        ccv_out = dpool.tile([4 * 2 * 128, T], BF16, tag="ccv_out")
                        nc.gpsimd.collective_compute(
                            "AllGather", mybir.AluOpType.bypass,
                            ins=[ccv_in.opt()], outs=[ccv_out.opt()],
                            replica_groups=GROUPS)
                        ccv_r = ccv_out.rearrange("(c s p) t -> p c s t", c=4, s=2)

                with nc.named_scope(f"layer{l}_attn"):
                    # o-proj weights: issue DMA early so it overlaps attention
                    wot = wopool.tile([128, KT, KT, 128], BF16, tag="w_o")
                    nc.sync.dma_start(wot[:], wo_d[l][:])

                    kg = []
                    for g in range(KVH):
                        # duplicate k rows into both partition halves so matmuls
                        # with q heads at base 0 or 64 both have matching bases
                        kgt = apool.tile([128, 4, T], BF16, tag=f"kg{g}")
                        src = cck_r[64 * (g % 2):64 * (g % 2) + 64, :, g // 2, :]
                        nc.sync.dma_start(kgt[0:64], src)
                        nc.sync.dma_start(kgt[64:128], src)
                        kg.append(kgt)
                    # all gathered V in one contiguous SBUF tile; attention
                    # slices it directly as the stationary operand
                    vall = apool.tile([128, 4, 2, T], BF16, tag="vall")
                    nc.sync.dma_start(vall[:], ccv_r[:])

                    oT = apool.tile([128, KT, T], BF16, tag="oT")
                    with (
                        tc.tile_pool(name="psatt", bufs=2, space="PSUM") as pa,
                        tc.tile_pool(name="pssum", bufs=2, space="PSUM") as pas,
                        tc.tile_pool(name="pexp", bufs=5) as epool,
                    ):
                        # heads processed in pairs (2*hp, 2*hp+1) living on
                        # partition bases 0/64: their score matmuls target
                        # different PE row-groups, so interleaving them lets
                        # the array run both concurrently (LDW pull-ahead).
                        for hp in range(NH // 2):
                            g = hp // 2
                            pjs = {0: [], 1: []}
                            for c2 in range(2):
                                pss = {}
                                for hh in (0, 1):
                                    pss[hh] = pa.tile([128, 4, T], F32, tag="ps_s", name=f"ps_s{hh}")
                                for jj in range(4):
                                    c = 2 * c2 + jj // 2
                                    mt = jj % 2
                                    for hh in (0, 1):
                                        base = 64 * hh
                                        nc.tensor.matmul(
                                            pss[hh][:, jj, :],
                                            kg[g][base:base + 64, c,
                                                  mt * 128:(mt + 1) * 128],
                                            qT[base:base + 64, hp, :],
                                            start=True, stop=True)
                                for hh in (0, 1):
                                    e1 = epool.tile([128, 4, T], BF16, tag="e1")
                                    nc.scalar.activation(e1[:], pss[hh][:], AF.Exp,
                                                         scale=0.125)
                                    pj = epool.tile([128, 4, T], BF16, tag="pj")
                                    nc.vector.tensor_mul(
                                        out=pj[:], in0=e1[:],
                                        in1=mask_sb[:, 4 * c2:4 * c2 + 4, :])
                                    pjs[hh].append(pj)
                            for hh in (0, 1):
                                base = 64 * hh
                                ps_sum = pas.tile([128, T], F32, tag="ps_sum")
                                ps_o = pas.tile([64, T], F32, tag="ps_o")
                                for j in range(8):
                                    c = j // 2
                                    tt = j % 2
                                    pj_sl = pjs[hh][c // 2][:, (c % 2) * 2 + tt, :]
                                    nc.tensor.matmul(ps_sum[:], ones_sb[:], pj_sl,
                                                     start=(j == 0), stop=(j == 7))
                                for j in range(8):
                                    c = j // 2
                                    tt = j % 2
                                    pj_sl = pjs[hh][c // 2][:, (c % 2) * 2 + tt, :]
                                    nc.tensor.matmul(
                                        ps_o[:],
                                        vall[:, c, tt, g * HD:(g + 1) * HD],
                                        pj_sl,
                                        start=(j == 0), stop=(j == 7))
                                invb = epool.tile([128, T], F32, tag="invb")
                                nc.vector.reciprocal_approx_fast(out=invb[:],
                                                                 in_=ps_sum[:])
                                nc.vector.tensor_mul(out=oT[base:base + 64, hp, :],
                                                     in0=ps_o[:], in1=invb[0:64, :])

                    # ---- o-projection + residual ----
                    with tc.tile_pool(name="psoproj", bufs=2, space="PSUM") as po:
                        for m in range(KT):
                            ps = po.tile([128, T], F32, tag="ps_op")
                            for j in range(KT):
                                nc.tensor.matmul(ps[:], wot[:, j, m], oT[:, j, :],
                                                 start=(j == 0), stop=(j == KT - 1))
                            nc.vector.tensor_add(out=xT[:, m, :], in0=xT[:, m, :],
                                                 in1=ps[:])

                with nc.named_scope(f"layer{l}_mlp"):
                    h2T = rmsnorm(xT)
                    with (
                        tc.tile_pool(name="psmlpd", bufs=1, space="PSUM") as pmd,
                        tc.tile_pool(name="psmlp", bufs=2, space="PSUM") as pm,
                    ):
                        ps_d = [pmd.tile([128, 2, T], F32, tag=f"ps_d{i}", name=f"ps_d{i}")
                                for i in range(4)]
                        for c in range(NC2):
                            wmt = wmpool.tile([128, 6144], BF16, tag="wmlp")
                            nc.sync.dma_start(wmt[:], wm_d[l][c])
                            for fi in range(2):
                                ps_g = pm.tile([128, T], F32, tag="ps_g")
                                for kt in range(KT):
                                    off = GOFF + (kt * 2 + fi) * 128
                                    nc.tensor.matmul(ps_g[:], wmt[:, off:off + 128],
                                                     h2T[:, kt],
                                                     start=(kt == 0), stop=(kt == KT - 1))
                                ps_u = pm.tile([128, T], F32, tag="ps_u")
                                for kt in range(KT):
                                    off = UOFF + (kt * 2 + fi) * 128
                                    nc.tensor.matmul(ps_u[:], wmt[:, off:off + 128],
                                                     h2T[:, kt],
                                                     start=(kt == 0), stop=(kt == KT - 1))
                                silu = tpool.tile([128, T], F32, tag="silu")
                                nc.scalar.activation(silu[:], ps_g[:], AF.Silu)
                                gu = tpool.tile([128, T], BF16, tag="gu")
                                nc.vector.tensor_mul(out=gu[:], in0=silu[:], in1=ps_u[:])
                                first = (c == 0 and fi == 0)
                                last = (c == NC2 - 1 and fi == 1)
                                for m in range(KT):
                                    off = DOFF + (fi * 8 + m) * 128
                                    # start=True clears the WHOLE bank's has_written,
                                    # so only the first matmul touching each bank may
                                    # set it; the odd slice's first write then stores
                                    # (has_written=0) and later writes accumulate.
                                    nc.tensor.matmul(ps_d[m // 2][:, m % 2, :],
                                                     wmt[:, off:off + 128], gu[:],
                                                     start=(first and m % 2 == 0),
                                                     stop=last,
                                                     skip_group_check=True)
                        for m in range(KT):
                            nc.vector.tensor_add(out=xT[:, m, :], in0=xT[:, m, :],
                                                 in1=ps_d[m // 2][:, m % 2, :])

            for p in reversed(layer_scope):
                p.__exit__(None, None, None)

            # =================== LM head ===================
            with nc.named_scope("lm_head"):
                hfT = rmsnorm(xT)
                cc2_in = dpool.tile([H, T], BF16, tag="cc2_in")
                nc.scalar.dma_start(cc2_in.rearrange("(kt p) t -> p kt t", p=128),
                                    hfT[:])
                cc2_out = dpool.tile([4 * H, T], BF16, tag="cc2_out")
                nc.gpsimd.collective_compute(
                    "AllGather", mybir.AluOpType.bypass,
                    ins=[cc2_in.opt()], outs=[cc2_out.opt()],
                    replica_groups=GROUPS)
                cc2_r = cc2_out.rearrange("(c kt p) t -> p c kt t", c=4, kt=KT)

                with (
                    tc.tile_pool(name="hall", bufs=1) as hallp,
                    tc.tile_pool(name="embp", bufs=3) as embp,
                    tc.tile_pool(name="lsbp", bufs=2) as lsbp,
                    tc.tile_pool(name="pslm", bufs=4, space="PSUM") as plm,
                ):
                    ha = []
                    for m8 in range(8):
                        hat = hallp.tile([128, KT, 128], BF16, tag=f"ha{m8}")
                        nc.scalar.dma_start(
                            hat[:],
                            cc2_r[:, m8 // 2, :, 128 * (m8 % 2):128 * (m8 % 2) + 128])
                        ha.append(hat)
                    for vc in range(NVC):
                        et = embp.tile([128, KT, VC], BF16, tag="emb")
                        nc.sync.dma_start(et[:], embT[vc])
                        lsb = lsbp.tile([128, 8, VC], F32, tag="lsb")
                        for m8 in range(8):
                            ps = plm.tile([128, VC], F32, tag="ps_lm")
                            for kt in range(KT):
                                nc.tensor.matmul(ps[:], ha[m8][:, kt], et[:, kt],
                                                 start=(kt == 0), stop=(kt == KT - 1))
                            nc.vector.tensor_copy(out=lsb[:, m8, :], in_=ps[:])
                        nc.sync.dma_start(logits_t[vc], lsb[:])

    nc.finalize()
    return nc


# ---------------- host side ----------------

def _host_prep(inputs):
    """Build per-core input maps from full inputs (tile-ready bf16 layouts)."""
    ids = np.asarray(inputs["input_ids"])
    embed = np.asarray(inputs["embed"], dtype=np.float32)
    n1 = np.asarray(inputs["norm1_w"], dtype=np.float32)
    n2 = np.asarray(inputs["norm2_w"], dtype=np.float32)
    nf = np.asarray(inputs["final_norm_w"], dtype=np.float32)

    inv_freq = 1.0 / (THETA ** (np.arange(0, HD, 2, dtype=np.float64) / HD))
    R64 = np.zeros((HD, HD), np.float32)
    for i in range(32):
        R64[i, i + 32] = -1.0
        R64[i + 32, i] = 1.0
    Rblk = np.zeros((128, 128), np.float32)
    Rblk[:64, :64] = R64
    Rblk[64:, 64:] = R64
    ropeR = np.ascontiguousarray(Rblk.T).astype(BF16NP)
    ones128 = np.ones((128, 128), BF16NP)

    def bf(x):
        return np.ascontiguousarray(x).astype(BF16NP)

    common = {"ropeR": ropeR, "ones_in": ones128}
    for l in range(L):
        wq = n1[l][:, None] * np.asarray(inputs["wq"][l], np.float32)    # [H, H]
        common[f"wq{l}"] = bf(wq.reshape(KT, 128, KT, 128).transpose(1, 2, 0, 3))
        wk = n1[l][:, None] * np.asarray(inputs["wk"][l], np.float32)    # [H, 256]
        common[f"wk{l}"] = bf(wk.reshape(KT, 128, 2, 128).transpose(1, 2, 0, 3))
        wv = n1[l][:, None] * np.asarray(inputs["wv"][l], np.float32)    # [H, 256]
        common[f"wv{l}"] = bf(wv.reshape(KT, 128, 256).transpose(1, 0, 2))
        wo = np.asarray(inputs["wo"][l], np.float32)                     # [H, H]
        # in-feature index = 64*(2j+s)+d -> partition p = 64*s+d, free j
        common[f"wo{l}"] = bf(wo.reshape(KT, 2, 64, KT, 128).transpose(1, 2, 0, 3, 4)
                              .reshape(128, KT, KT, 128))
        wg = n2[l][:, None] * np.asarray(inputs["w_gate"][l], np.float32)  # [H, I]
        wu = n2[l][:, None] * np.asarray(inputs["w_up"][l], np.float32)
        wd = np.asarray(inputs["w_down"][l], np.float32)                   # [I, H]
        gpart = wg.reshape(KT, 128, NC2, 2, 128).transpose(2, 1, 0, 3, 4) \
            .reshape(NC2, 128, 2048)
        upart = wu.reshape(KT, 128, NC2, 2, 128).transpose(2, 1, 0, 3, 4) \
            .reshape(NC2, 128, 2048)
        dpart = wd.reshape(NC2, 2, 128, KT, 128).transpose(0, 2, 1, 3, 4) \
            .reshape(NC2, 128, 2048)
        common[f"wm{l}"] = bf(np.concatenate([gpart, upart, dpart], axis=2))

    in_maps = []
    for core in range(NCORE):
        b, qc = core // 4, core % 4
        pos = np.arange(T, dtype=np.float64) + qc * T
        freqs = np.outer(pos, inv_freq)
        emb = np.concatenate([freqs, freqs], axis=-1)
        cosT = np.cos(emb).T.astype(np.float32)     # [64, T]
        sinT = np.sin(emb).T.astype(np.float32)
        qpos = (np.arange(T) + qc * T)
        kvpos = (np.arange(8)[None, :] * 128 + np.arange(128)[:, None])  # [128, 8]
        mask = (kvpos[:, :, None] <= qpos[None, None, :]).astype(BF16NP)
        x0T = embed[ids[b, qc * T:(qc + 1) * T]].T.astype(np.float32)    # [H, T]
        x0T = np.ascontiguousarray(x0T.reshape(KT, 128, T).transpose(1, 0, 2))
        vbase = qc * VSH
        embs = (nf[:, None] * embed[vbase:vbase + VSH].T)                # [H, VSH]
        embs = bf(embs.reshape(KT, 128, NVC, VC).transpose(2, 1, 0, 3))
        m = dict(common)
        m.update({
            "x0T": x0T,
            "cos2": np.ascontiguousarray(np.tile(cosT, (2, 1))),
            "sin2": np.ascontiguousarray(np.tile(sinT, (2, 1))),
            "mask": np.ascontiguousarray(mask),
            "embT": embs,
        })
        in_maps.append(m)
    return in_maps


def _get_program():
    if "prog" not in _CACHE:
        _CACHE["prog"] = build_program()
    return _CACHE["prog"]


def run(inputs, debug_layers=False, trace=False):
    nc = _get_program()
    in_maps = _host_prep(inputs)
    res = run_bass_kernel_spmd(nc, in_maps, core_ids=list(range(NCORE)), trace=trace)
    out = np.zeros((B, S, V), np.float32)
    for b in range(B):
        for qc in range(4):
            lt = res.results[4 * b + qc]["logits_t"]     # [NVC, 128, 8, VC]
            shard = lt.transpose(2, 1, 0, 3).reshape(S, VSH)
            out[b, :, qc * VSH:(qc + 1) * VSH] = shard
    return out, res


def kernel(**inputs) -> np.ndarray:
    out, _ = run(inputs)
    return out


# revision 18
# speedup vs baseline: 1.6533x; 1.0040x over previous
"""Trainium2 Bass kernel for a 4-layer GQA transformer LM (nn_CustomLLM_35278861369705).

Sharding: sequence-parallel across 8 cores - 2 batch groups x 4 sequence chunks
of 256 tokens. Activations kept transposed [feature, token] on device.
All matmul operands are bf16 (fp32 PSUM accumulation, fp32 residual stream).
Weights are pre-laid out on the host in tile-ready form so each weight tensor
loads with a single large fully-contiguous DMA; MLP weights stream in 16
chunks of 1.5MB (gate+up+down packed) with triple buffering.
Per layer: RMSNorm (ones-matmul partition reduction), fused-rope QKV,
split K / V group AllGathers (K gathered while Q/V project), masked full-kv
attention, SwiGLU MLP with PSUM-resident down-proj accumulators.
Final AllGather of hidden states + vocab-sharded tied LM head with SBUF-staged
logits written back in 2MB tiles.
"""
import numpy as np
import ml_dtypes

import concourse.bass as bass
import concourse.mybir as mybir
import concourse.tile as tile
from concourse import bacc
from concourse.bass_utils import run_bass_kernel_spmd

V, H, NH, KVH, I, L, S, B = 32000, 1024, 16, 4, 4096, 4, 1024, 2
HD = 64
THETA = 10000.0
EPS = 1e-5
T = 256            # tokens per core
NCORE = 8
GROUPS = [[0, 1, 2, 3], [4, 5, 6, 7]]
VSH = V // 4       # vocab shard per core (within its 4-core group)
KT = H // 128      # 8
NC2 = 16           # MLP I-chunks per layer (256 feats each)
NVC = 16           # vocab chunks per core
VC = VSH // NVC    # 500

F32 = mybir.dt.float32
BF16 = mybir.dt.bfloat16
AF = mybir.ActivationFunctionType
BF16NP = ml_dtypes.bfloat16

# MLP chunk free-dim offsets (elements within the flat [128, 6144] tile)
GOFF = 0           # gate region:  (kt, fi) -> (kt*2+fi)*128
UOFF = 2048        # up region:    (kt, fi) -> UOFF + (kt*2+fi)*128
DOFF = 4096        # down region:  (ci, m)  -> DOFF + (ci*8+m)*128

_CACHE = {}


def build_program():
    nc = bacc.Bacc("TRN2", target_bir_lowering=False, debug=False,
                   num_devices=NCORE)

    # ---------------- I/O ----------------
    x0T = nc.dram_tensor("x0T", [128, KT, T], F32, kind="ExternalInput").ap()
    cos2 = nc.dram_tensor("cos2", [128, T], F32, kind="ExternalInput").ap()
    sin2 = nc.dram_tensor("sin2", [128, T], F32, kind="ExternalInput").ap()
    ropeR = nc.dram_tensor("ropeR", [128, 128], BF16, kind="ExternalInput").ap()
    ones_in = nc.dram_tensor("ones_in", [128, 128], BF16, kind="ExternalInput").ap()
    mask_in = nc.dram_tensor("mask", [128, 8, T], BF16, kind="ExternalInput").ap()
    embT = nc.dram_tensor("embT", [NVC, 128, KT, VC], BF16, kind="ExternalInput").ap()
    wq_d, wk_d, wv_d, wo_d, wm_d = [], [], [], [], []
    for l in range(L):
        wq_d.append(nc.dram_tensor(f"wq{l}", [128, KT, KT, 128], BF16,
                                   kind="ExternalInput").ap())   # [p, m, kt, of]
        wk_d.append(nc.dram_tensor(f"wk{l}", [128, 2, KT, 128], BF16,
                                   kind="ExternalInput").ap())   # [p, m, kt, of]
        wv_d.append(nc.dram_tensor(f"wv{l}", [128, KT, 256], BF16,
                                   kind="ExternalInput").ap())   # [p, kt, of]
        wo_d.append(nc.dram_tensor(f"wo{l}", [128, KT, KT, 128], BF16,
                                   kind="ExternalInput").ap())   # [p, j, m, of]
        wm_d.append(nc.dram_tensor(f"wm{l}", [NC2, 128, 6144], BF16,
                                   kind="ExternalInput").ap())   # packed g/u/d
    logits_t = nc.dram_tensor("logits_t", [NVC, 128, 8, VC], F32,
                              kind="ExternalOutput").ap()

    with tile.TileContext(nc) as tc:
        with (
            tc.tile_pool(name="const", bufs=1) as cpool,
            tc.tile_pool(name="xres", bufs=1) as xpool,
            tc.tile_pool(name="hnorm", bufs=2) as hpool,
            tc.tile_pool(name="sqp", bufs=2) as sqpool,
            tc.tile_pool(name="tmps", bufs=3) as tpool,
            tc.tile_pool(name="dram", bufs=2, space="DRAM") as dpool,
        ):
            # ---- residual stream + first-needed constants first ----
            xT = xpool.tile([128, KT, T], F32, tag="xT")
            nc.sync.dma_start(xT[:], x0T[:])
            ones_sb = cpool.tile([128, 128], BF16, tag="ones")
            nc.sync.dma_start(ones_sb[:], ones_in[:])
            cos_sb = cpool.tile([128, T], F32, tag="cos")
            sin_sb = cpool.tile([128, T], F32, tag="sin")
            nc.sync.dma_start(cos_sb[:], cos2[:])
            nc.sync.dma_start(sin_sb[:], sin2[:])
            ropeR_sb = cpool.tile([128, 128], BF16, tag="ropeR")
            nc.sync.dma_start(ropeR_sb[:], ropeR[:])
            mask_sb = cpool.tile([128, 8, T], BF16, tag="mask")
            nc.sync.dma_start(mask_sb[:], mask_in[:])

            def rmsnorm(src):
                """src: [128, KT, T] f32 -> hT [128, KT, T] bf16 (norm weights
                are folded into the following matmul weights on host).
                Per-kt pipelined so PE sum-matmuls overlap the DVE squares."""
                sq = sqpool.tile([128, KT, T], BF16, tag="sq")
                with tc.tile_pool(name="psnorm", bufs=1, space="PSUM") as pp:
                    ps = pp.tile([128, T], F32, tag="ps_norm")
                    for kt in range(KT):
                        nc.vector.tensor_mul(out=sq[:, kt], in0=src[:, kt],
                                             in1=src[:, kt])
                        nc.tensor.matmul(ps[:], ones_sb[:], sq[:, kt],
                                         start=(kt == 0), stop=(kt == KT - 1))
                    ms = tpool.tile([128, T], F32, tag="ms")
                    nc.scalar.activation(ms[:], ps[:], AF.Copy, bias=EPS, scale=1.0 / H)
                rcp = tpool.tile([128, T], F32, tag="rcp")
                nc.vector.reciprocal_approx_fast(out=rcp[:], in_=ms[:])
                inv = tpool.tile([128, T], F32, tag="inv")
                nc.scalar.activation(inv[:], rcp[:], AF.Sqrt)
                hT = hpool.tile([128, KT, T], BF16, tag="h")
                for kt in range(KT):
                    nc.vector.tensor_mul(out=hT[:, kt], in0=src[:, kt],
                                         in1=inv[:])
                return hT

            # =================== layers ===================
            layer_scope = (
                tc.tile_pool(name="wqp", bufs=1),
                tc.tile_pool(name="wkp", bufs=2),
                tc.tile_pool(name="wvp", bufs=2),
                tc.tile_pool(name="wop", bufs=1),
                tc.tile_pool(name="wmp", bufs=3),
                tc.tile_pool(name="acts", bufs=1),
            )
            wqpool, wkpool, wvpool, wopool, wmpool, apool = \
                [p.__enter__() for p in layer_scope]
            for l in range(L):
                with nc.named_scope(f"layer{l}_qkv"):
                    hT = rmsnorm(xT)
                    # single big weight DMAs (fully contiguous layouts);
                    # wk first: k-proj gates the K AllGather critical path
                    wkt = wkpool.tile([128, 2, KT, 128], BF16, tag="w_k")
                    nc.sync.dma_start(wkt[:], wk_d[l][:])
                    wqt = wqpool.tile([128, KT, KT, 128], BF16, tag="w_q")
                    nc.sync.dma_start(wqt[:], wq_d[l][:])
                    wvt = wvpool.tile([128, KT, 256], BF16, tag="w_v")
                    nc.sync.dma_start(wvt[:], wv_d[l][:])

                    qT = apool.tile([128, KT, T], BF16, tag="qT")
                    kT_loc = apool.tile([128, 2, T], BF16, tag="kT_loc")
                    v_loc = apool.tile([128, 2, T], BF16, tag="v_loc")

                    with tc.tile_pool(name="psqkv", bufs=2, space="PSUM") as pq:
                        def proj_rope(wt_sl, out_sl):
                            """one 128-feature projection slice + rope -> out_sl."""
                            ps = pq.tile([128, T], F32, tag="ps_qkv")
                            for kt in range(KT):
                                nc.tensor.matmul(ps[:], wt_sl[:, kt], hT[:, kt],
                                                 start=(kt == 0), stop=(kt == KT - 1))
                            raw = tpool.tile([128, T], BF16, tag="qraw")
                            nc.vector.tensor_copy(out=raw[:], in_=ps[:])
                            rot = pq.tile([128, T], F32, tag="ps_rot")
                            nc.tensor.matmul(rot[:], ropeR_sb[:], raw[:],
                                             start=True, stop=True)
                            tcs = tpool.tile([128, T], F32, tag="tcos")
                            nc.vector.tensor_mul(out=tcs[:], in0=ps[:], in1=cos_sb[:])
                            tsn = tpool.tile([128, T], F32, tag="tsin")
                            nc.vector.tensor_mul(out=tsn[:], in0=rot[:], in1=sin_sb[:])
                            nc.vector.tensor_add(out=out_sl, in0=tcs[:], in1=tsn[:])

                        # K first so its AllGather overlaps Q/V compute
                        for m in range(2):
                            proj_rope(wkt[:, m], kT_loc[:, m, :])
                        cck_in = dpool.tile([2 * 128, T], BF16, tag="cck_in")
                        nc.sync.dma_start(
                            cck_in.rearrange("(s p) t -> p s t", p=128), kT_loc[:])
                        cck_out = dpool.tile([4 * 2 * 128, T], BF16, tag="cck_out")
                        nc.gpsimd.collective_compute(
                            "AllGather", mybir.AluOpType.bypass,
                            ins=[cck_in.opt()], outs=[cck_out.opt()],
                            replica_groups=GROUPS)
                        cck_r = cck_out.rearrange("(c s p) t -> p c s t", c=4, s=2)

                        for m in range(KT):
                            proj_rope(wqt[:, m], qT[:, m, :])

                        # V in natural [token, feature] layout
                        for tt in range(2):
                            psv = pq.tile([128, T], F32, tag="ps_qkv")
                            for kt in range(KT):
                                nc.tensor.matmul(psv[:], hT[:, kt, tt * 128:(tt + 1) * 128],
                                                 wvt[:, kt],
                                                 start=(kt == 0), stop=(kt == KT - 1))
                            nc.vector.tensor_copy(out=v_loc[:, tt, :], in_=psv[:])
                        ccv_in = dpool.tile([2 * 128, T], BF16, tag="ccv_in")
                        nc.sync.dma_start(
                            ccv_in.rearrange("(s p) t -> p s t", p=128), v_loc[:])
                        ccv_out = dpool.tile([4 * 2 * 128, T], BF16, tag="ccv_out")
                        nc.gpsimd.collective_compute(
                            "AllGather", mybir.AluOpType.bypass,
                            ins=[ccv_in.opt()], outs=[ccv_out.opt()],
                            replica_groups=GROUPS)
                        ccv_r = ccv_out.rearrange("(c s p) t -> p c s t", c=4, s=2)

                with nc.named_scope(f"layer{l}_attn"):
                    # o-proj weights: issue DMA early so it overlaps attention
                    wot = wopool.tile([128, KT, KT, 128], BF16, tag="w_o")
                    nc.sync.dma_start(wot[:], wo_d[l][:])

                    kg = []
                    for g in range(KVH):
                        # duplicate k rows into both partition halves so matmuls
                        # with q heads at base 0 or 64 both have matching bases
                        kgt = apool.tile([128, 4, T], BF16, tag=f"kg{g}")
                        src = cck_r[64 * (g % 2):64 * (g % 2) + 64, :, g // 2, :]
                        nc.sync.dma_start(kgt[0:64], src)
                        nc.sync.dma_start(kgt[64:128], src)
                        kg.append(kgt)
                    # all gathered V in one contiguous SBUF tile; attention
                    # slices it directly as the stationary operand
                    vall = apool.tile([128, 4, 2, T], BF16, tag="vall")
                    nc.sync.dma_start(vall[:], ccv_r[:])

                    oT = apool.tile([128, KT, T], BF16, tag="oT")
                    with (
                        tc.tile_pool(name="psatt", bufs=2, space="PSUM") as pa,
                        tc.tile_pool(name="pssum", bufs=2, space="PSUM") as pas,
                        tc.tile_pool(name="pexp", bufs=5) as epool,
                    ):
                        # heads processed in pairs (2*hp, 2*hp+1) living on
                        # partition bases 0/64: their score matmuls target
                        # different PE row-groups, so interleaving them lets
                        # the array run both concurrently (LDW pull-ahead).
                        for hp in range(NH // 2):
                            g = hp // 2
                            pjs = {0: [], 1: []}
                            for c2 in range(2):
                                pss = {}
                                for hh in (0, 1):
                                    pss[hh] = pa.tile([128, 4, T], F32, tag="ps_s", name=f"ps_s{hh}")
                                for jj in range(4):
                                    c = 2 * c2 + jj // 2
                                    mt = jj % 2
                                    for hh in (0, 1):
                                        base = 64 * hh
                                        nc.tensor.matmul(
                                            pss[hh][:, jj, :],
                                            kg[g][base:base + 64, c,
                                                  mt * 128:(mt + 1) * 128],
                                            qT[base:base + 64, hp, :],
                                            start=True, stop=True)
                                for hh in (0, 1):
                                    e1 = epool.tile([128, 4, T], BF16, tag="e1")
                                    nc.scalar.activation(e1[:], pss[hh][:], AF.Exp,
                                                         scale=0.125)
                                    pj = epool.tile([128, 4, T], BF16, tag="pj")
                                    nc.vector.tensor_mul(
                                        out=pj[:], in0=e1[:],
                                        in1=mask_sb[:, 4 * c2:4 * c2 + 4, :])
                                    pjs[hh].append(pj)
                            for hh in (0, 1):
                                base = 64 * hh
                                ps_sum = pas.tile([128, T], F32, tag="ps_sum")
                                ps_o = pas.tile([64, T], F32, tag="ps_o")
                                for j in range(8):
                                    c = j // 2
                                    tt = j % 2
                                    pj_sl = pjs[hh][c // 2][:, (c % 2) * 2 + tt, :]
                                    nc.tensor.matmul(ps_sum[:], ones_sb[:], pj_sl,
                                                     start=(j == 0), stop=(j == 7))
                                for j in range(8):
                                    c = j // 2
                                    tt = j % 2
                                    pj_sl = pjs[hh][c // 2][:, (c % 2) * 2 + tt, :]
                                    nc.tensor.matmul(
                                        ps_o[:],
                                        vall[:, c, tt, g * HD:(g + 1) * HD],
                                        pj_sl,
                                        start=(j == 0), stop=(j == 7))
                                invb = epool.tile([128, T], F32, tag="invb")
                                nc.vector.reciprocal_approx_fast(out=invb[:],
                                                                 in_=ps_sum[:])
                                nc.vector.tensor_mul(out=oT[base:base + 64, hp, :],
                                                     in0=ps_o[:], in1=invb[0:64, :])

                    # ---- o-projection + residual ----
                    with tc.tile_pool(name="psoproj", bufs=2, space="PSUM") as po:
                        for m in range(KT):
                            ps = po.tile([128, T], F32, tag="ps_op")
                            for j in range(KT):
                                nc.tensor.matmul(ps[:], wot[:, j, m], oT[:, j, :],
                                                 start=(j == 0), stop=(j == KT - 1))
                            nc.vector.tensor_add(out=xT[:, m, :], in0=xT[:, m, :],
                                                 in1=ps[:])

                with nc.named_scope(f"layer{l}_mlp"):
                    h2T = rmsnorm(xT)
                    with (
                        tc.tile_pool(name="psmlpd", bufs=1, space="PSUM") as pmd,
                        tc.tile_pool(name="psmlp", bufs=2, space="PSUM") as pm,
                    ):
                        ps_d = [pmd.tile([128, 2, T], F32, tag=f"ps_d{i}", name=f"ps_d{i}")
                                for i in range(4)]
                        for c in range(NC2):
                            wmt = wmpool.tile([128, 6144], BF16, tag="wmlp")
                            nc.sync.dma_start(wmt[:], wm_d[l][c])
                            for fi in range(2):
                                ps_g = pm.tile([128, T], F32, tag="ps_g")
                                for kt in range(KT):
                                    off = GOFF + (kt * 2 + fi) * 128
                                    nc.tensor.matmul(ps_g[:], wmt[:, off:off + 128],
                                                     h2T[:, kt],
                                                     start=(kt == 0), stop=(kt == KT - 1))
                                ps_u = pm.tile([128, T], F32, tag="ps_u")
                                for kt in range(KT):
                                    off = UOFF + (kt * 2 + fi) * 128
                                    nc.tensor.matmul(ps_u[:], wmt[:, off:off + 128],
                                                     h2T[:, kt],
                                                     start=(kt == 0), stop=(kt == KT - 1))
                                silu = tpool.tile([128, T], F32, tag="silu")
                                nc.scalar.activation(silu[:], ps_g[:], AF.Silu)
                                gu = tpool.tile([128, T], BF16, tag="gu")
                                nc.vector.tensor_mul(out=gu[:], in0=silu[:], in1=ps_u[:])
                                first = (c == 0 and fi == 0)
                                last = (c == NC2 - 1 and fi == 1)
                                for m in range(KT):
                                    off = DOFF + (fi * 8 + m) * 128
                                    # start=True clears the WHOLE bank's has_written,
                                    # so only the first matmul touching each bank may
                                    # set it; the odd slice's first write then stores
                                    # (has_written=0) and later writes accumulate.
                                    nc.tensor.matmul(ps_d[m // 2][:, m % 2, :],
                                                     wmt[:, off:off + 128], gu[:],
                                                     start=(first and m % 2 == 0),
                                                     stop=last,
                                                     skip_group_check=True)
                        for m in range(KT):
                            nc.vector.tensor_add(out=xT[:, m, :], in0=xT[:, m, :],
                                                 in1=ps_d[m // 2][:, m % 2, :])

            for p in reversed(layer_scope):
                p.__exit__(None, None, None)

            # =================== LM head ===================
            with nc.named_scope("lm_head"):
                hfT = rmsnorm(xT)
                cc2_in = dpool.tile([H, T], BF16, tag="cc2_in")
                nc.scalar.dma_start(cc2_in.rearrange("(kt p) t -> p kt t", p=128),
                                    hfT[:])
                cc2_out = dpool.tile([4 * H, T], BF16, tag="cc2_out")
                nc.gpsimd.collective_compute(
                    "AllGather", mybir.AluOpType.bypass,
                    ins=[cc2_in.opt()], outs=[cc2_out.opt()],
                    replica_groups=GROUPS)
                cc2_r = cc2_out.rearrange("(c kt p) t -> p c kt t", c=4, kt=KT)

                with (
                    tc.tile_pool(name="hall", bufs=1) as hallp,
                    tc.tile_pool(name="embp", bufs=3) as embp,
                    tc.tile_pool(name="lsbp", bufs=2) as lsbp,
                    tc.tile_pool(name="pslm", bufs=4, space="PSUM") as plm,
                ):
                    ha = []
                    for m8 in range(8):
                        hat = hallp.tile([128, KT, 128], BF16, tag=f"ha{m8}")
                        nc.scalar.dma_start(
                            hat[:],
                            cc2_r[:, m8 // 2, :, 128 * (m8 % 2):128 * (m8 % 2) + 128])
                        ha.append(hat)
                    for vc in range(NVC):
                        et = embp.tile([128, KT, VC], BF16, tag="emb")
                        nc.sync.dma_start(et[:], embT[vc])
                        lsb = lsbp.tile([128, 8, VC], F32, tag="lsb")
                        for m8 in range(8):
                            ps = plm.tile([128, VC], F32, tag="ps_lm")
                            for kt in range(KT):
                                nc.tensor.matmul(ps[:], ha[m8][:, kt], et[:, kt],
                                                 start=(kt == 0), stop=(kt == KT - 1))
                            nc.vector.tensor_copy(out=lsb[:, m8, :], in_=ps[:])
                        nc.sync.dma_start(logits_t[vc], lsb[:])

    nc.finalize()
    return nc


# ---------------- host side ----------------

def _host_prep(inputs):
    """Build per-core input maps from full inputs (tile-ready bf16 layouts)."""
    ids = np.asarray(inputs["input_ids"])
    embed = np.asarray(inputs["embed"], dtype=np.float32)
    n1 = np.asarray(inputs["norm1_w"], dtype=np.float32)
    n2 = np.asarray(inputs["norm2_w"], dtype=np.float32)
    nf = np.asarray(inputs["final_norm_w"], dtype=np.float32)

    inv_freq = 1.0 / (THETA ** (np.arange(0, HD, 2, dtype=np.float64) / HD))
    R64 = np.zeros((HD, HD), np.float32)
    for i in range(32):
        R64[i, i + 32] = -1.0
        R64[i + 32, i] = 1.0
    Rblk = np.zeros((128, 128), np.float32)
    Rblk[:64, :64] = R64
    Rblk[64:, 64:] = R64
    ropeR = np.ascontiguousarray(Rblk.T).astype(BF16NP)
    ones128 = np.ones((128, 128), BF16NP)

    def bf(x):
        return np.ascontiguousarray(x).astype(BF16NP)

    common = {"ropeR": ropeR, "ones_in": ones128}
    for l in range(L):
        wq = n1[l][:, None] * np.asarray(inputs["wq"][l], np.float32)    # [H, H]
        common[f"wq{l}"] = bf(wq.reshape(KT, 128, KT, 128).transpose(1, 2, 0, 3))
        wk = n1[l][:, None] * np.asarray(inputs["wk"][l], np.float32)    # [H, 256]
        common[f"wk{l}"] = bf(wk.reshape(KT, 128, 2, 128).transpose(1, 2, 0, 3))
        wv = n1[l][:, None] * np.asarray(inputs["wv"][l], np.float32)    # [H, 256]
        common[f"wv{l}"] = bf(wv.reshape(KT, 128, 256).transpose(1, 0, 2))
        wo = np.asarray(inputs["wo"][l], np.float32)                     # [H, H]
        # in-feature index = 64*(2j+s)+d -> partition p = 64*s+d, free j
        common[f"wo{l}"] = bf(wo.reshape(KT, 2, 64, KT, 128).transpose(1, 2, 0, 3, 4)
                              .reshape(128, KT, KT, 128))
        wg = n2[l][:, None] * np.asarray(inputs["w_gate"][l], np.float32)  # [H, I]
        wu = n2[l][:, None] * np.asarray(inputs["w_up"][l], np.float32)
        wd = np.asarray(inputs["w_down"][l], np.float32)                   # [I, H]
        gpart = wg.reshape(KT, 128, NC2, 2, 128).transpose(2, 1, 0, 3, 4) \
            .reshape(NC2, 128, 2048)
        upart = wu.reshape(KT, 128, NC2, 2, 128).transpose(2, 1, 0, 3, 4) \
            .reshape(NC2, 128, 2048)
        dpart = wd.reshape(NC2, 2, 128, KT, 128).transpose(0, 2, 1, 3, 4) \
            .reshape(NC2, 128, 2048)
        common[f"wm{l}"] = bf(np.concatenate([gpart, upart, dpart], axis=2))

    in_maps = []
    for core in range(NCORE):
        b, qc = core // 4, core % 4
        pos = np.arange(T, dtype=np.float64) + qc * T
        freqs = np.outer(pos, inv_freq)
        emb = np.concatenate([freqs, freqs], axis=-1)
        cosT = np.cos(emb).T.astype(np.float32)     # [64, T]
        sinT = np.sin(emb).T.astype(np.float32)
        qpos = (np.arange(T) + qc * T)
        kvpos = (np.arange(8)[None, :] * 128 + np.arange(128)[:, None])  # [128, 8]
        mask = (kvpos[:, :, None] <= qpos[None, None, :]).astype(BF16NP)
        x0T = embed[ids[b, qc * T:(qc + 1) * T]].T.astype(np.float32)    # [H, T]
        x0T = np.ascontiguousarray(x0T.reshape(KT, 128, T).transpose(1, 0, 2))
        vbase = qc * VSH
        embs = (nf[:, None] * embed[vbase:vbase + VSH].T)                # [H, VSH]
        embs = bf(embs.reshape(KT, 128, NVC, VC).transpose(2, 1, 0, 3))
        m = dict(common)
        m.update({
            "x0T": x0T,
            "cos2": np.ascontiguousarray(np.tile(cosT, (2, 1))),
            "sin2": np.ascontiguousarray(np.tile(sinT, (2, 1))),
            "mask": np.ascontiguousarray(mask),
            "embT": embs,
        })
        in_maps.append(m)
    return in_maps


def _get_program():
    if "prog" not in _CACHE:
        _CACHE["prog"] = build_program()
    return _CACHE["prog"]


def run(inputs, debug_layers=False, trace=False):
    nc = _get_program()
    in_maps = _host_prep(inputs)
    res = run_bass_kernel_spmd(nc, in_maps, core_ids=list(range(NCORE)), trace=trace)
    out = np.zeros((B, S, V), np.float32)
    for b in range(B):
        for qc in range(4):
            lt = res.results[4 * b + qc]["logits_t"]     # [NVC, 128, 8, VC]
            shard = lt.transpose(2, 1, 0, 3).reshape(S, VSH)
            out[b, :, qc * VSH:(qc + 1) * VSH] = shard
    return out, res


def kernel(**inputs) -> np.ndarray:
    out, _ = run(inputs)
    return out
